# revision 45
# baseline (speedup 1.0000x reference)
"""Trainium2 Bass kernel for the Ewald energy nn.Module.

Math restructuring (validated to ~2.6e-4 rel err against the jax reference
with the fp16 fast path):
  E = E_real + E_recip with the charge contraction folded in:
    E_real  = 0.5*COEF * sum_s sum_ij q_i q_j (erf(r/(sqrt2*gam_ij)) - erf(r/(sqrt2*eta)))/r
              over the symmetric half of the 6x6 grid of 128-atom tile pairs
              (off-diagonal pairs weighted 2x).  d^2 comes from Gram-matrix
              matmuls on the TensorEngine; atoms are pre-sorted along x so
              (tile-pair, x-shift-group) units whose x-gap exceeds the
              real-space cutoff are culled exactly (the reference masks
              r>cutoff).  With the true invgamma[i,i] = 1/(2 sigma_i), the
              eps-regularised (i==i, s=0) pairs reproduce the analytic
              self-energy term exactly, so no separate self part exists.
    E_recip = 0.5*COEF*(4pi/V) * sum_k w_k * (Sc_k^2 + Ss_k^2), structure
              factors via PE matmuls; only the ~1646 nonzero-weight k of the
              17^3 grid are computed.  theta is range-reduced via fractional
              coordinates and the (y + 1.5*2^23) - 1.5*2^23 round trick so
              Sin stays inside its [-pi, pi] table range.

Sharding: surviving real-space units are distributed round-robin over the 8
cores (UPC slots each, dummies zero-weighted); active k split 256 per core;
every core returns a scalar partial and the host sums the 8.

The run path is latency-dominated (axon tunnel RTT ~35 ms, measured
wall-clock), so the kernel minimises per-call host work and bytes shipped:
  * ~50 KB/core of inputs.  Redundant tensors are rebuilt on device: the
    9-shift -2b rows from a per-unit base row + shift constants (broadcast
    add), |b|^2 = 0.25*colsum((-2b)^2) via a K=3 ones matmul, invgamma from
    sigma^2 vectors via a K=2 outer-sum matmul + Abs_reciprocal_sqrt, q_j
    broadcast over shifts in the final reduce, and the constant rows of the
    theta operands (memset row 0 - compute-engine partition ranges must
    start at 0/32/64).
  * d^2 accumulates K=3 (-2 a_i . b) + K=1 (+|a_i|^2) + K=1 (+|b|^2)
    matmuls in psum; the 1e-8 floor rides the activation bias so it is
    added AFTER the Gram cancellation (folding it into |a_i|^2 loses it to
    fp32 rounding and yields 1/sqrt(0) = inf on the diagonal).
  * _run_spmd caches an AOT-compiled jitted shard_map executable per
    program (the library helper re-traces, re-lowers and re-runs DVE-table
    generation on EVERY call, ~200 ms of host work).

ScalarEngine activations are phase-ordered (Abs_reciprocal_sqrt -> Erf ->
Sin) via emission order plus explicit scheduler edges through two tiny ACT
barrier copies, so each ACT table set loads exactly once.  The erf pipeline
runs in fp16 to unlock the DVE 2x perf mode.
"""
import math
import os
import sys
import numpy as np

_TRN_REPO = "/opt/trn_rl_repo"
if _TRN_REPO not in sys.path and os.path.isdir(_TRN_REPO):
    sys.path.insert(0, _TRN_REPO)

COEF = 14.399645478425668
N = 768
NT = 6            # 128-atom tiles
P = 128
NSHIFT_REAL = 1   # -> 27 shifts, 3 x-groups of 9
NSG = 3
SG = 9
UCOL = P * SG     # 1152 columns per unit
CH = 384          # matmul chunk
CHS = 512         # psum chunk stride (bank aligned)
NCH = 3
NCORES = 8
NSHIFT_RECIP = 8  # -> 17^3 = 4913 k-vectors; ~1646 carry weight
KPC = 256         # active k per core
_MAX_UNITS = 48   # provable upper bound on surviving units

_nc_cache = {}


def _tile_pairs():
    return [(ti, tj) for ti in range(NT) for tj in range(ti, NT)]


def _select_units(a, shifts, cutoff):
    """Cull (tilepair, shift-group) units whose x-gap exceeds the real-space
    cutoff.  Requires atoms sorted by x; sorted blocks guarantee at most 48
    survivors."""
    x = a[:, 0]
    lo = [x[t * P:(t + 1) * P].min() for t in range(NT)]
    hi = [x[t * P:(t + 1) * P].max() for t in range(NT)]
    units = []
    for (ti, tj) in _tile_pairs():
        for sg in range(NSG):
            keep = False
            for s in range(SG):
                sx = float(shifts[sg * SG + s, 0])
                d_lo = lo[tj] + sx - hi[ti]
                d_hi = hi[tj] + sx - lo[ti]
                if not (d_lo > cutoff or d_hi < -cutoff):
                    keep = True
                    break
            if keep:
                units.append((ti, tj, sg))
    assert len(units) <= _MAX_UNITS, f"{len(units)} units > {_MAX_UNITS}"
    return units


def build_program(c1, upc, dbg=False):
    """Build + compile the per-core Bass program (same on all cores).

    c1 = 1/(sqrt(2)*eta) is baked in as the erf scale constant; upc is the
    number of real-space unit slots per core.
    """
    key = ("nc", round(float(c1), 12), int(upc), bool(dbg))
    if key in _nc_cache:
        return _nc_cache[key]

    import concourse.bacc as bacc
    import concourse.tile as tile
    from concourse import mybir
    from concourse.tile import add_dep_helper

    AF = mybir.ActivationFunctionType
    OP = mybir.AluOpType
    f32 = mybir.dt.float32
    f16 = mybir.dt.float16

    nc = bacc.Bacc("TRN2", target_bir_lowering=False, debug=False)

    dt_in = {}

    def din(name, shape, dtype=f32):
        dt_in[name] = nc.dram_tensor(name, shape, dtype,
                                     kind="ExternalInput").ap()
        return dt_in[name]

    din("lhsu", [3, upc * P])          # a_i rows per unit
    din("a2r", [1, upc * P])           # |a_i|^2 per unit (single row)
    din("rbase", [3, upc * P])         # -2*a_j rows per unit
    din("rshift", [3, upc * SG])       # -2*shift rows per unit
    din("s2l", [2, upc * P])           # [sigma_i^2; 1] per unit
    din("s2r", [2, upc * P])           # [1; sigma_j^2] per unit
    din("q2c", [P, upc], f16)
    din("qjr", [upc, P])               # q_j per unit (broadcast over shifts)
    din("mT", [3, 2 * KPC])            # k rows; row 3 (24.0/24.25) on device
    din("fT", [3, N])                  # frac rows; ones row 3 on device
    din("w2", [2, KPC])
    din("qmat16", [P, NT], f16)
    out_d = nc.dram_tensor("out", [1, 1], f32, kind="ExternalOutput").ap()
    if dbg:
        dbg_d = {
            "dbg_invg": nc.dram_tensor("dbg_invg", [P, upc * P], f16,
                                       kind="ExternalOutput").ap(),
            "dbg_rall": nc.dram_tensor("dbg_rall", [P, upc * UCOL], f16,
                                       kind="ExternalOutput").ap(),
            "dbg_arows": nc.dram_tensor("dbg_arows", [upc, UCOL], f32,
                                        kind="ExternalOutput").ap(),
            "dbg_rr": nc.dram_tensor("dbg_rr", [upc, 1], f32,
                                     kind="ExternalOutput").ap(),
            "dbg_rk": nc.dram_tensor("dbg_rk", [2, 1], f32,
                                     kind="ExternalOutput").ap(),
            "dbg_scs": nc.dram_tensor("dbg_scs", [2, KPC], f32,
                                      kind="ExternalOutput").ap(),
        }

    with tile.TileContext(nc) as tc:
        with tc.tile_pool(name="consts", bufs=1) as cp, \
             tc.tile_pool(name="rall", bufs=1) as rp, \
             tc.tile_pool(name="trig", bufs=6) as tp_, \
             tc.tile_pool(name="work", bufs=2) as wp, \
             tc.tile_pool(name="rbuild", bufs=2) as rbp, \
             tc.tile_pool(name="stage", bufs=2) as stp, \
             tc.tile_pool(name="scps", bufs=1, space="PSUM") as scp, \
             tc.tile_pool(name="bigps", bufs=2, space="PSUM") as bigp, \
             tc.tile_pool(name="thps", bufs=1, space="PSUM") as thp:

            # ---- load inputs to SBUF ----
            # fT/mT get a constant row 0 prepended on device (memset must
            # start at partition 0; DMA lands the data on partitions 1-3)
            sb = {}
            for name, ap in dt_in.items():
                if name in ("fT", "mT"):
                    t = cp.tile([4, ap.shape[1]], ap.dtype, name=f"sb_{name}")
                    nc.sync.dma_start(t[1:4, :], ap[:])
                else:
                    t = cp.tile(list(ap.shape), ap.dtype, name=f"sb_{name}")
                    nc.sync.dma_start(t[:], ap[:])
                sb[name] = t
            nc.vector.memset(sb["fT"][0:1, :], 1.0)
            nc.vector.memset(sb["mT"][0:1, 0:KPC], 24.0)
            nc.vector.memset(sb["mT"][0:1, KPC:], 24.25)

            beps = cp.tile([P, 1], f32)
            nc.vector.memset(beps[:], 1e-8)
            ones_t = cp.tile([P, 1], f32)
            nc.vector.memset(ones_t[:], 1.0)
            onesP = cp.tile([1, P], f32)           # K=1 lhs for the b^2 matmul
            nc.vector.memset(onesP[:], 1.0)
            onesC = cp.tile([1, CH], f32)          # K=1 rhs for the a^2 matmul
            nc.vector.memset(onesC[:], 1.0)
            ones3 = cp.tile([3, 1], f32)           # K=3 lhs for the b^2 colsum
            nc.vector.memset(ones3[:], 1.0)

            rall = rp.tile([P, upc * UCOL], f16)   # 1/r for all units
            invg_sb = cp.tile([P, upc * P], f16)   # 1/sqrt(2(si^2+sj^2))
            arows = cp.tile([upc, UCOL], f32)      # per-unit reduced rows
            scs = cp.tile([2, KPC], f32)           # [Sc; Ss] (squared later)
            scs_st = cp.tile([1, 2 * KPC], f32)    # partition-0 staging

            # ============ recip theta + range reduction (early) ============
            scargs = []
            RC = 12582912.0  # 1.5 * 2**23: (y + RC) - RC == round-to-nearest
            for t in range(NT):
                th = thp.tile([P, 2 * KPC], f32, tag="th")
                nc.tensor.matmul(th[:, 0:KPC], sb["fT"][:, t * P:(t + 1) * P],
                                 sb["mT"][:, 0:KPC], start=True, stop=True)
                nc.tensor.matmul(th[:, KPC:], sb["fT"][:, t * P:(t + 1) * P],
                                 sb["mT"][:, KPC:], start=True, stop=True)
                scarg = tp_.tile([P, 2 * KPC], f32, tag="scarg")
                rnd = stp.tile([P, 2 * KPC], f32, tag="rnd")
                nc.vector.tensor_scalar(rnd[:], th[:], RC, RC,
                                        OP.add, OP.subtract)
                nc.vector.tensor_tensor(scarg[:], th[:], rnd[:], OP.subtract)
                scargs.append(scarg)

            # ============ real part, phase 1: invg + d2 + 1/r ============
            # invg[i,j] = 1/sqrt(2(si^2+sj^2)) from a K=2 outer-sum matmul;
            # the true 1/(2*sigma_i) diagonal makes the eps-regularised
            # (i==i, s=0) pair reproduce the analytic self-energy term
            # exactly, so no separate self part is needed.
            sqrt_instrs = []
            for u in range(upc):
                ig = scp.tile([P, P], f32, tag="sc")
                nc.tensor.matmul(ig[:], sb["s2l"][:, u * P:(u + 1) * P],
                                 sb["s2r"][:, u * P:(u + 1) * P],
                                 start=True, stop=True)
                si = nc.scalar.activation(invg_sb[:, u * P:(u + 1) * P],
                                          ig[:], AF.Abs_reciprocal_sqrt,
                                          bias=0.0, scale=2.0)
                if sqrt_instrs:
                    add_dep_helper(si.ins, sqrt_instrs[-1].ins, sync=False,
                                   reason="sqrt unit order")
                sqrt_instrs.append(si)
            # -2b rows built on device from per-unit base + shift (broadcast
            # add), |b|^2 as 0.25 * colsum((-2b)^2) via a K=3 ones matmul;
            # then per 384-chunk: K=3 Gram (-2 a_i . b) + K=1 (+|a_i|^2)
            # + K=1 (+|b|^2) accumulated in psum, drained by one strided
            # Abs_reciprocal_sqrt with a pure-eps bias (the eps must be
            # added after the full cancellation or fp32 rounding eats it)
            for u in range(upc):
                rhs3 = rbp.tile([3, UCOL], f32, tag="rhs3")
                nc.vector.tensor_tensor(
                    rhs3[:].rearrange("p (s j) -> p s j", s=SG),
                    sb["rbase"][:, u * P:(u + 1) * P].unsqueeze(1)
                        .broadcast_to([3, SG, P]),
                    sb["rshift"][:, u * SG:(u + 1) * SG].unsqueeze(2)
                        .broadcast_to([3, SG, P]),
                    OP.add)
                sq3 = rbp.tile([3, UCOL], f32, tag="sq3")
                nc.vector.tensor_tensor(sq3[:], rhs3[:], rhs3[:], OP.mult)
                b2ps = bigp.tile([1, NCH * CHS], f32, tag="big")
                for ch in range(NCH):
                    nc.tensor.matmul(b2ps[0:1, ch * CHS:ch * CHS + CH],
                                     ones3[:], sq3[:, ch * CH:(ch + 1) * CH],
                                     start=True, stop=True)
                b2row = rbp.tile([1, UCOL], f32, tag="b2r")
                nc.vector.tensor_scalar_mul(
                    b2row[:].rearrange("p (c f) -> p c f", c=NCH),
                    b2ps[:].rearrange("p (c f) -> p c f", c=NCH)[:, :, 0:CH],
                    0.25)
                d2 = bigp.tile([P, NCH * CHS], f32, tag="big")
                for ch in range(NCH):
                    nc.tensor.matmul(d2[:, ch * CHS:ch * CHS + CH],
                                     sb["lhsu"][:, u * P:(u + 1) * P],
                                     rhs3[:, ch * CH:(ch + 1) * CH],
                                     start=True, stop=False)
                    nc.tensor.matmul(d2[:, ch * CHS:ch * CHS + CH],
                                     sb["a2r"][0:1, u * P:(u + 1) * P],
                                     onesC[:], start=False, stop=False)
                    nc.tensor.matmul(d2[:, ch * CHS:ch * CHS + CH],
                                     onesP[:],
                                     b2row[0:1, ch * CH:(ch + 1) * CH],
                                     start=False, stop=True)
                si = nc.scalar.activation(
                    rall[:, u * UCOL:(u + 1) * UCOL]
                        .rearrange("p (c f) -> p c f", c=NCH),
                    d2[:].rearrange("p (c f) -> p c f", c=NCH)[:, :, 0:CH],
                    AF.Abs_reciprocal_sqrt, bias=beps[:], scale=1.0)
                add_dep_helper(si.ins, sqrt_instrs[-1].ins, sync=False,
                               reason="sqrt unit order")
                sqrt_instrs.append(si)

            # ---- ACT barrier 1: abs_rsqrt -> erf ----
            bar1t = cp.tile([1, 1], f32)
            b1 = nc.scalar.copy(bar1t[:], ones_t[0:1, :])
            for s in sqrt_instrs:
                add_dep_helper(b1.ins, s.ins, sync=False, reason="act sqrt->erf")

            # ============ real part, phase 2: erf pipeline (fp16) ============
            erf_instrs = []
            for u in range(upc):
                rinv_u = rall[:, u * UCOL:(u + 1) * UCOL]
                H = UCOL // 2
                r_u = wp.tile([P, UCOL], f16, tag="r")
                with nc.allow_low_precision(reason="fp16 erf pipeline"):
                    nc.vector.reciprocal(r_u[:, 0:H], rinv_u[:, 0:H])
                    nc.vector.reciprocal(r_u[:, H:], rinv_u[:, H:])
                erf1 = wp.tile([P, UCOL], f16, tag="erf1")
                for hs in (slice(0, H), slice(H, UCOL)):
                    e1 = nc.scalar.activation(erf1[:, hs], r_u[:, hs], AF.Erf,
                                              bias=0.0, scale=float(c1))
                    add_dep_helper(e1.ins, b1.ins, sync=False,
                                   reason="act sqrt->erf")
                    erf_instrs.append(e1)
                arg2 = wp.tile([P, UCOL], f16, tag="arg2")
                # columns are s-major (col = s*128 + j): broadcast invg over s
                # with the unit-stride j innermost so DVE 2x mode applies
                invg_b = invg_sb[:, u * P:(u + 1) * P].unsqueeze(1) \
                    .broadcast_to([P, SG, P])
                nc.vector.tensor_tensor(
                    arg2[:].rearrange("p (s j) -> p s j", s=SG),
                    r_u[:].rearrange("p (s j) -> p s j", s=SG),
                    invg_b, OP.mult)
                erf2 = wp.tile([P, UCOL], f16, tag="erf2")
                e2 = nc.scalar.activation(erf2[:], arg2[:], AF.Erf,
                                          bias=0.0, scale=1.0)
                add_dep_helper(e2.ins, b1.ins, sync=False, reason="act sqrt->erf")
                erf_instrs.append(e2)
                # diff and valr in place (erf2 <- erf2-erf1 on GPSIMD,
                # erf1 <- diff*rinv on DVE)
                nc.gpsimd.tensor_tensor(erf2[:, 0:H], erf2[:, 0:H],
                                        erf1[:, 0:H], OP.subtract)
                nc.vector.tensor_tensor(erf2[:, H:], erf2[:, H:],
                                        erf1[:, H:], OP.subtract)
                nc.vector.tensor_tensor(erf1[:], erf2[:], rinv_u, OP.mult)
                ast = stp.tile([1, UCOL], f32, tag="ast")
                red = bigp.tile([1, NCH * CHS], f32, tag="big")
                for ch in range(NCH):
                    nc.tensor.matmul(red[0:1, ch * CHS:ch * CHS + CH],
                                     sb["q2c"][:, u:u + 1],
                                     erf1[:, ch * CH:(ch + 1) * CH],
                                     start=True, stop=True)
                nc.vector.tensor_copy(
                    ast[:].rearrange("p (c f) -> p c f", c=NCH),
                    red[:].rearrange("p (c f) -> p c f", c=NCH)[:, :, 0:CH])
                nc.sync.dma_start(arows[u:u + 1, :], ast[:])

            racc_r = cp.tile([upc, 1], f32)
            trash_r = cp.tile([upc, UCOL], f32)
            nc.vector.tensor_tensor(
                trash_r[:].rearrange("u (s j) -> u s j", s=SG),
                arows[:].rearrange("u (s j) -> u s j", s=SG),
                sb["qjr"][:].unsqueeze(1).broadcast_to([upc, SG, P]),
                OP.mult)
            nc.vector.tensor_reduce(racc_r[:], trash_r[:],
                                    axis=mybir.AxisListType.X, op=OP.add)

            # ---- ACT barrier 2: erf -> sin ----
            bar2t = cp.tile([1, 1], f32)
            b2 = nc.scalar.copy(bar2t[:], ones_t[0:1, :])
            for e in erf_instrs:
                add_dep_helper(b2.ins, e.ins, sync=False, reason="act erf->sin")

            # ============ reciprocal part ============
            # psum row: [Sc | Ss] accumulators in one bank
            scrow = scp.tile([1, 2 * KPC], f32, tag="sc")
            sncs = []
            for t in range(NT):
                snc = tp_.tile([P, 2 * KPC], f16, tag="snc")
                si = nc.scalar.activation(snc[:], scargs[t][:], AF.Sin,
                                          bias=0.0, scale=float(2 * math.pi))
                add_dep_helper(si.ins, b2.ins, sync=False, reason="act erf->sin")
                sncs.append(snc)
            for t in range(NT):
                nc.tensor.matmul(scrow[0:1, 0:KPC], sb["qmat16"][:, t:t + 1],
                                 sncs[t][:, KPC:],
                                 start=(t == 0), stop=(t == NT - 1))
            for t in range(NT):
                nc.tensor.matmul(scrow[0:1, KPC:], sb["qmat16"][:, t:t + 1],
                                 sncs[t][:, 0:KPC],
                                 start=(t == 0), stop=(t == NT - 1))
            nc.scalar.copy(scs_st[:], scrow[:])
            nc.sync.dma_start(scs[:], scs_st[:])

            sqk = cp.tile([2, KPC], f32)
            nc.vector.tensor_tensor(sqk[:], scs[:], scs[:], OP.mult)
            racc_k = cp.tile([2, 1], f32)
            trash_k = cp.tile([2, KPC], f32)
            nc.vector.tensor_tensor(trash_k[:], sqk[:], sb["w2"][:], OP.mult)
            nc.vector.tensor_reduce(racc_k[:], trash_k[:],
                                    axis=mybir.AxisListType.X, op=OP.add)

            # ============ combine ============
            # (no separate self part: the eps-regularised diagonal pairs of
            # the real part reproduce it exactly)
            sacc = cp.tile([P, 1], f32)
            nc.vector.memset(sacc[:], 0.0)
            nc.vector.tensor_tensor(sacc[0:upc, :], sacc[0:upc, :],
                                    racc_r[:], OP.add)
            nc.vector.tensor_tensor(sacc[0:2, :], sacc[0:2, :],
                                    racc_k[:], OP.add)
            fin = thp.tile([1, 1], f32, tag="th")
            nc.tensor.matmul(fin[:], sacc[:], ones_t[:], start=True, stop=True)
            outt = cp.tile([1, 1], f32)
            nc.vector.tensor_copy(outt[:], fin[:])
            nc.sync.dma_start(out_d[:], outt[:])
            if dbg:
                nc.sync.dma_start(dbg_d["dbg_invg"][:], invg_sb[:])
                nc.sync.dma_start(dbg_d["dbg_rall"][:], rall[:])
                nc.sync.dma_start(dbg_d["dbg_arows"][:], arows[:])
                nc.sync.dma_start(dbg_d["dbg_rr"][:], racc_r[:])
                nc.sync.dma_start(dbg_d["dbg_rk"][:], racc_k[:])
                nc.sync.dma_start(dbg_d["dbg_scs"][:], scs[:])

    nc.compile()
    _nc_cache[key] = nc
    return nc


def _shift_grid(n):
    r = np.arange(-n, n + 1, dtype=np.float64)
    g = np.stack(np.meshgrid(r, r, r, indexing="ij"), axis=-1)
    return g.reshape(-1, 3)


def prep_in_maps(pos, cell, charges, sigma_table, species_idx):
    """Host-side shard prep: returns (in_maps list of 8 dicts, c1, upc)."""
    pos = np.asarray(pos, np.float32)
    cell = np.asarray(cell, np.float32)
    if cell.ndim == 3:
        cell = cell[0]
    q = np.asarray(charges, np.float32).reshape(-1)
    sigma_table = np.asarray(sigma_table, np.float32)
    species_idx = np.asarray(species_idx).astype(np.int64)
    sigmas = sigma_table[species_idx]

    vol = abs(np.linalg.det(cell.astype(np.float64)))
    eta = (vol ** 2 / N) ** (1.0 / 6.0) / math.sqrt(2.0 * math.pi)
    cutoff_recip = math.sqrt(-2.0 * math.log(1e-8)) / eta
    cutoff_real = math.sqrt(-2.0 * math.log(1e-8)) * eta
    c1 = 1.0 / (math.sqrt(2.0) * eta)

    # sort atoms along x so the 128-atom tiles become x-slabs (enables exact
    # culling of far tile-pair/shift units)
    perm = np.argsort(pos[:, 0], kind="stable")
    pos = pos[perm]
    q = q[perm]
    sigmas = sigmas[perm]

    center = 0.5 * cell.astype(np.float64).sum(axis=0)
    a = (pos.astype(np.float64) - center).astype(np.float32)
    a2 = (a * a).sum(1).astype(np.float32)
    shifts = (_shift_grid(NSHIFT_REAL) @ cell.astype(np.float64)).astype(np.float32)

    sig2 = sigmas.astype(np.float32) ** 2

    units = _select_units(a, shifts, cutoff_real)
    upc = max(1, (len(units) + NCORES - 1) // NCORES)
    units = units + [None] * (NCORES * upc - len(units))

    # reciprocal k-grid: keep only k with nonzero weight (exact culling)
    gk = _shift_grid(NSHIFT_RECIP)                     # (4913, 3) float64
    recip = 2.0 * math.pi * np.linalg.inv(cell.astype(np.float64)).T
    ks_all = gk @ recip
    klen_all = np.linalg.norm(ks_all, axis=-1)
    kmask = (klen_all > 1e-8) & (klen_all < cutoff_recip)
    kidx = np.nonzero(kmask)[0]
    KTOT = NCORES * KPC
    assert len(kidx) <= KTOT, f"{len(kidx)} active k > {KTOT} slots"
    gk_pad = np.zeros((KTOT, 3), np.float64)
    gk_pad[: len(kidx)] = gk[kidx]
    wk = np.zeros(KTOT, np.float64)
    wk[: len(kidx)] = (np.exp(-0.5 * (eta * klen_all[kidx]) ** 2)
                       / klen_all[kidx] ** 2)
    wk = wk * (0.5 * COEF * 4.0 * math.pi / vol)
    frac = pos.astype(np.float64) @ np.linalg.inv(cell.astype(np.float64))
    fT_all = frac.T.astype(np.float32)                 # (3, N)

    # no separate self part: with invg_ii = 1/(2 sigma_i) the device's
    # eps-regularised (i==i, s=0) pairs give (erf(r invg) - erf(r c1))/r
    # -> (2/sqrt(pi))(1/(2 sigma_i) - 1/(sqrt2 eta))
    #  = 1/(sqrt(pi) sigma_i) - sqrt(2/pi)/eta, the analytic self term.

    in_maps = []
    for c in range(NCORES):
        lhsu = np.zeros((3, upc * P), np.float32)
        a2r = np.zeros((1, upc * P), np.float32)
        rbase = np.zeros((3, upc * P), np.float32)
        rshift = np.zeros((3, upc * SG), np.float32)
        s2l = np.ones((2, upc * P), np.float32)
        s2r = np.ones((2, upc * P), np.float32)
        q2c = np.zeros((P, upc), np.float32)
        qjr = np.zeros((upc, P), np.float32)
        for k in range(upc):
            unit = units[c * upc + k]
            if unit is None:
                continue   # zero-weight dummy; s2u stays 1 -> finite invg
            ti, tj, sg = unit
            wu = 1.0 if ti == tj else 2.0
            ai = a[ti * P:(ti + 1) * P]                # (128, 3)
            lhsu[:, k * P:(k + 1) * P] = ai.T
            a2r[0, k * P:(k + 1) * P] = a2[ti * P:(ti + 1) * P]
            aj = a[tj * P:(tj + 1) * P]                # (128, 3)
            # device builds -2b[s,j] = -2 a_j + -2 shift_s (s-major columns)
            rbase[:, k * P:(k + 1) * P] = -2.0 * aj.T
            rshift[:, k * SG:(k + 1) * SG] = \
                -2.0 * shifts[sg * SG:(sg + 1) * SG].T
            s2l[0, k * P:(k + 1) * P] = sig2[ti * P:(ti + 1) * P]
            s2r[1, k * P:(k + 1) * P] = sig2[tj * P:(tj + 1) * P]
            q2c[:, k] = q[ti * P:(ti + 1) * P] * np.float32(0.5 * COEF * wu)
            qjr[k] = q[tj * P:(tj + 1) * P]
        ksl = slice(c * KPC, (c + 1) * KPC)
        mTc = np.empty((3, 2 * KPC), np.float32)
        mTc[:, 0:KPC] = gk_pad[ksl].T.astype(np.float32)
        mTc[:, KPC:] = mTc[:, 0:KPC]
        w2c = np.broadcast_to(wk[ksl].astype(np.float32), (2, KPC)).copy()
        in_maps.append({
            "lhsu": lhsu, "a2r": a2r, "rbase": rbase, "rshift": rshift,
            "s2l": s2l, "s2r": s2r,
            "q2c": q2c.astype(np.float16), "qjr": qjr,
            "mT": mTc, "fT": fT_all.copy(), "w2": w2c,
            "qmat16": q.reshape(NT, P).T.astype(np.float16),
        })
    return in_maps, c1, upc


_runner_cache = {}


def _make_runner(nc, n_cores):
    """Build the jitted SPMD executable for ``nc`` ONCE.

    ``bass_utils.run_bass_kernel_spmd`` (axon path) creates a fresh closure
    and a fresh ``jax.jit`` on every call, so every invocation re-traces,
    re-lowers and re-runs ``compile_bir_kernel``/DVE-table generation
    (~200 ms of host work per call).  This mirrors its exact execution
    semantics (same ``_bass_exec_p`` bind, same shard_map layout, same
    donated zero-initialised outputs) but hoists all of that out of the
    per-call path: steady-state calls are just concat + dispatch + fetch.
    """
    import jax
    from concourse import bass2jax, mybir

    bass2jax.install_neuronx_cc_hook()
    if nc.dbg_addr is not None and nc.dbg_callbacks:
        raise RuntimeError("dbg callbacks unsupported in cached runner")
    partition_name = nc.partition_id_tensor.name if nc.partition_id_tensor else None
    dbg_name = nc.dbg_addr.name if nc.dbg_addr is not None else None

    in_names, in_specs_np, out_names, out_avals = [], [], [], []
    for alloc in nc.m.functions[0].allocations:
        if not isinstance(alloc, mybir.MemoryLocationSet):
            continue
        name = alloc.memorylocations[0].name
        if alloc.kind == "ExternalInput":
            if name != partition_name:
                in_names.append(name)
                in_specs_np.append((tuple(alloc.tensor_shape),
                                    mybir.dt.np(alloc.dtype)))
        elif alloc.kind == "ExternalOutput":
            out_names.append(name)
            out_avals.append(jax.core.ShapedArray(
                tuple(alloc.tensor_shape), mybir.dt.np(alloc.dtype)))
    n_params = len(in_names)
    n_outs = len(out_names)
    all_names = in_names + out_names + ([partition_name] if partition_name else [])
    donate = tuple(range(n_params, n_params + n_outs))

    def _body(*args):
        operands = list(args)
        if partition_name is not None:
            operands.append(bass2jax.partition_id_tensor())
        return tuple(bass2jax._bass_exec_p.bind(
            *operands, out_avals=tuple(out_avals), in_names=tuple(all_names),
            out_names=tuple(out_names), lowering_input_output_aliases=(),
            sim_require_finite=True, sim_require_nnan=True, nc=nc))

    devices = jax.devices()[:n_cores]
    assert len(devices) == n_cores
    mesh = bass2jax.Mesh(np.asarray(devices), ("core",))
    PS = bass2jax.PartitionSpec
    sharded = jax.jit(
        bass2jax.shard_map(_body, mesh=mesh,
                           in_specs=(PS("core"),) * (n_params + n_outs),
                           out_specs=(PS("core"),) * n_outs,
                           check_rep=False),
        donate_argnums=donate, keep_unused=True)
    # AOT-compile to skip the python pjit dispatch path (~3-8 ms/call)
    try:
        structs = [jax.ShapeDtypeStruct((n_cores * s[0], *s[1:]), dt)
                   for s, dt in in_specs_np]
        structs += [jax.ShapeDtypeStruct((n_cores * av.shape[0],
                                          *av.shape[1:]), av.dtype)
                    for av in out_avals]
        call = sharded.lower(*structs).compile()
    except Exception:
        call = sharded

    def run(in_maps):
        if dbg_name is not None:
            in_maps = [{**m, dbg_name: np.zeros((1, 2), np.uint32)}
                       for m in in_maps]
        concat_in = [
            np.concatenate([np.asarray(m[name]) for m in in_maps], axis=0)
            for name in in_names]
        concat_zeros = [
            np.zeros((n_cores * av.shape[0], *av.shape[1:]), av.dtype)
            for av in out_avals]
        out_arrs = call(*concat_in, *concat_zeros)
        outs_np = [np.asarray(o) for o in out_arrs]
        return [{name: outs_np[i].reshape(n_cores, *out_avals[i].shape)[c]
                 for i, name in enumerate(out_names)}
                for c in range(n_cores)]

    return run


def _run_spmd(nc, in_maps):
    """Run ``nc`` on 8 cores; cached-jit fast path with library fallback."""
    key = id(nc)
    run = _runner_cache.get(key)
    if run is None:
        try:
            run = _make_runner(nc, NCORES)
            _runner_cache[key] = run
        except Exception:
            run = None
    if run is not None:
        try:
            return run(in_maps)
        except Exception:
            _runner_cache.pop(key, None)
    from concourse import bass_utils
    res = bass_utils.run_bass_kernel_spmd(nc, in_maps,
                                          core_ids=list(range(NCORES)))
    return res.results


def kernel(pos, cell, charges, sigma_table, species_idx,
           nshift_real, nshift_recip):
    assert int(nshift_real) == NSHIFT_REAL and int(nshift_recip) == NSHIFT_RECIP, \
        "kernel compiled for nshift_real=1, nshift_recip=8"
    pos = np.asarray(pos)
    assert pos.shape == (N, 3)

    in_maps, c1, upc = prep_in_maps(pos, cell, charges, sigma_table,
                                    species_idx)
    nc = build_program(c1, upc)

    results = _run_spmd(nc, in_maps)
    e = np.float64(0.0)
    for i in range(NCORES):
        e += np.float64(results[i]["out"][0, 0])
    return np.array([[e]], dtype=np.float32)



# revision 47
# speedup vs baseline: 1.0036x; 1.0036x over previous
"""Trainium2 Bass kernel for the Ewald energy nn.Module.

Math restructuring (validated to ~2.6e-4 rel err against the jax reference
with the fp16 fast path):
  E = E_real + E_recip with the charge contraction folded in:
    E_real  = 0.5*COEF * sum_s sum_ij q_i q_j (erf(r/(sqrt2*gam_ij)) - erf(r/(sqrt2*eta)))/r
              over the symmetric half of the 6x6 grid of 128-atom tile pairs
              (off-diagonal pairs weighted 2x).  d^2 comes from Gram-matrix
              matmuls on the TensorEngine; atoms are pre-sorted along x so
              (tile-pair, x-shift-group) units whose x-gap exceeds the
              real-space cutoff are culled exactly (the reference masks
              r>cutoff).  With the true invgamma[i,i] = 1/(2 sigma_i), the
              eps-regularised (i==i, s=0) pairs reproduce the analytic
              self-energy term exactly, so no separate self part exists.
    E_recip = 0.5*COEF*(4pi/V) * sum_k w_k * (Sc_k^2 + Ss_k^2), structure
              factors via PE matmuls; only the ~1646 nonzero-weight k of the
              17^3 grid are computed.  theta is range-reduced via fractional
              coordinates and the (y + 1.5*2^23) - 1.5*2^23 round trick so
              Sin stays inside its [-pi, pi] table range.

Sharding: surviving real-space units are distributed round-robin over the 8
cores (UPC slots each, dummies zero-weighted); active k split 256 per core;
every core returns a scalar partial and the host sums the 8.

The run path is latency-dominated (axon tunnel RTT ~35 ms, measured
wall-clock), so the kernel minimises per-call host work and bytes shipped:
  * ~50 KB/core of inputs.  Redundant tensors are rebuilt on device: the
    9-shift -2b rows from a per-unit base row + shift constants (broadcast
    add), |b|^2 = 0.25*colsum((-2b)^2) via a K=3 ones matmul, invgamma from
    sigma^2 vectors via a K=2 outer-sum matmul + Abs_reciprocal_sqrt, q_j
    broadcast over shifts in the final reduce, and the constant rows of the
    theta operands (memset row 0 - compute-engine partition ranges must
    start at 0/32/64).
  * d^2 accumulates K=3 (-2 a_i . b) + K=1 (+|a_i|^2) + K=1 (+|b|^2)
    matmuls in psum; the 1e-8 floor rides the activation bias so it is
    added AFTER the Gram cancellation (folding it into |a_i|^2 loses it to
    fp32 rounding and yields 1/sqrt(0) = inf on the diagonal).
  * _run_spmd caches an AOT-compiled jitted shard_map executable per
    program (the library helper re-traces, re-lowers and re-runs DVE-table
    generation on EVERY call, ~200 ms of host work).

ScalarEngine activations are phase-ordered (Abs_reciprocal_sqrt -> Erf ->
Sin) via emission order plus explicit scheduler edges through two tiny ACT
barrier copies, so each ACT table set loads exactly once.  The erf pipeline
runs in fp16 to unlock the DVE 2x perf mode.
"""
import math
import os
import sys
import numpy as np

_TRN_REPO = "/opt/trn_rl_repo"
if _TRN_REPO not in sys.path and os.path.isdir(_TRN_REPO):
    sys.path.insert(0, _TRN_REPO)

COEF = 14.399645478425668
N = 768
NT = 6            # 128-atom tiles
P = 128
NSHIFT_REAL = 1   # -> 27 shifts, 3 x-groups of 9
NSG = 3
SG = 9
UCOL = P * SG     # 1152 columns per unit
CH = 384          # matmul chunk
CHS = 512         # psum chunk stride (bank aligned)
NCH = 3
NCORES = 8
NSHIFT_RECIP = 8  # -> 17^3 = 4913 k-vectors; ~1646 carry weight
KPC = 256         # active k per core
_MAX_UNITS = 48   # provable upper bound on surviving units

_nc_cache = {}


def _tile_pairs():
    return [(ti, tj) for ti in range(NT) for tj in range(ti, NT)]


def _select_units(a, shifts, cutoff):
    """Cull (tilepair, shift-group) units whose x-gap exceeds the real-space
    cutoff.  Requires atoms sorted by x; sorted blocks guarantee at most 48
    survivors."""
    x = a[:, 0]
    lo = [x[t * P:(t + 1) * P].min() for t in range(NT)]
    hi = [x[t * P:(t + 1) * P].max() for t in range(NT)]
    units = []
    for (ti, tj) in _tile_pairs():
        for sg in range(NSG):
            keep = False
            for s in range(SG):
                sx = float(shifts[sg * SG + s, 0])
                d_lo = lo[tj] + sx - hi[ti]
                d_hi = hi[tj] + sx - lo[ti]
                if not (d_lo > cutoff or d_hi < -cutoff):
                    keep = True
                    break
            if keep:
                units.append((ti, tj, sg))
    assert len(units) <= _MAX_UNITS, f"{len(units)} units > {_MAX_UNITS}"
    return units


def build_program(c1, upc, dbg=False):
    """Build + compile the per-core Bass program (same on all cores).

    c1 = 1/(sqrt(2)*eta) is baked in as the erf scale constant; upc is the
    number of real-space unit slots per core.
    """
    key = ("nc", round(float(c1), 12), int(upc), bool(dbg))
    if key in _nc_cache:
        return _nc_cache[key]

    import concourse.bacc as bacc
    import concourse.tile as tile
    from concourse import mybir
    from concourse.tile import add_dep_helper

    AF = mybir.ActivationFunctionType
    OP = mybir.AluOpType
    f32 = mybir.dt.float32
    f16 = mybir.dt.float16

    nc = bacc.Bacc("TRN2", target_bir_lowering=False, debug=False)

    dt_in = {}

    def din(name, shape, dtype=f32):
        dt_in[name] = nc.dram_tensor(name, shape, dtype,
                                     kind="ExternalInput").ap()
        return dt_in[name]

    din("lhsu", [3, upc * P])          # a_i rows per unit
    din("a2r", [1, upc * P])           # |a_i|^2 per unit (single row)
    din("rbase", [3, upc * P])         # -2*a_j rows per unit
    din("rshift", [3, upc * SG])       # -2*shift rows per unit
    din("s2l", [2, upc * P])           # [sigma_i^2; 1] per unit
    din("s2r", [2, upc * P])           # [1; sigma_j^2] per unit
    din("q2c", [P, upc], f16)
    din("qjr", [upc, P])               # q_j per unit (broadcast over shifts)
    din("mT", [3, 2 * KPC])            # k rows; 24.0/24.25 row on device
    din("fT", [3, N])                  # frac rows; ones row on device
    din("w2", [2, KPC])
    din("qmat16", [P, NT], f16)
    out_d = nc.dram_tensor("out", [1, 1], f32, kind="ExternalOutput").ap()
    if dbg:
        dbg_d = {
            "dbg_invg": nc.dram_tensor("dbg_invg", [P, upc * P], f16,
                                       kind="ExternalOutput").ap(),
            "dbg_rall": nc.dram_tensor("dbg_rall", [P, upc * UCOL], f16,
                                       kind="ExternalOutput").ap(),
            "dbg_arows": nc.dram_tensor("dbg_arows", [upc, UCOL], f32,
                                        kind="ExternalOutput").ap(),
            "dbg_rr": nc.dram_tensor("dbg_rr", [upc, 1], f32,
                                     kind="ExternalOutput").ap(),
            "dbg_rk": nc.dram_tensor("dbg_rk", [2, 1], f32,
                                     kind="ExternalOutput").ap(),
            "dbg_scs": nc.dram_tensor("dbg_scs", [2, KPC], f32,
                                      kind="ExternalOutput").ap(),
        }

    with tile.TileContext(nc) as tc:
        with tc.tile_pool(name="consts", bufs=1) as cp, \
             tc.tile_pool(name="rall", bufs=1) as rp, \
             tc.tile_pool(name="trig", bufs=6) as tp_, \
             tc.tile_pool(name="work", bufs=2) as wp, \
             tc.tile_pool(name="rbuild", bufs=2) as rbp, \
             tc.tile_pool(name="stage", bufs=2) as stp, \
             tc.tile_pool(name="scps", bufs=1, space="PSUM") as scp, \
             tc.tile_pool(name="bigps", bufs=2, space="PSUM") as bigp, \
             tc.tile_pool(name="thps", bufs=1, space="PSUM") as thp:

            # ---- load inputs to SBUF ----
            # fT/mT get a constant row 0 prepended on device (memset must
            # start at partition 0; DMA lands the data on partitions 1-3)
            sb = {}
            for name, ap in dt_in.items():
                if name in ("fT", "mT"):
                    t = cp.tile([4, ap.shape[1]], ap.dtype, name=f"sb_{name}")
                    nc.sync.dma_start(t[1:4, :], ap[:])
                else:
                    t = cp.tile(list(ap.shape), ap.dtype, name=f"sb_{name}")
                    nc.sync.dma_start(t[:], ap[:])
                sb[name] = t
            nc.vector.memset(sb["fT"][0:1, :], 1.0)
            nc.vector.memset(sb["mT"][0:1, 0:KPC], 24.0)
            nc.vector.memset(sb["mT"][0:1, KPC:], 24.25)

            beps = cp.tile([P, 1], f32)
            nc.vector.memset(beps[:], 1e-8)
            ones_t = cp.tile([P, 1], f32)
            nc.vector.memset(ones_t[:], 1.0)
            onesP = cp.tile([1, P], f32)           # K=1 lhs for the b^2 matmul
            nc.vector.memset(onesP[:], 1.0)
            onesC = cp.tile([1, CH], f32)          # K=1 rhs for the a^2 matmul
            nc.vector.memset(onesC[:], 1.0)
            ones3 = cp.tile([3, 1], f32)           # K=3 lhs for the b^2 colsum
            nc.vector.memset(ones3[:], 1.0)

            rall = rp.tile([P, upc * UCOL], f16)   # 1/r for all units
            invg_sb = cp.tile([P, upc * P], f16)   # 1/sqrt(2(si^2+sj^2))
            arows = cp.tile([upc, UCOL], f32)      # per-unit reduced rows
            scs = cp.tile([2, KPC], f32)           # [Sc; Ss] (squared later)
            scs_st = cp.tile([1, 2 * KPC], f32)    # partition-0 staging

            # ============ recip theta + range reduction (early) ============
            scargs = []
            RC = 12582912.0  # 1.5 * 2**23: (y + RC) - RC == round-to-nearest
            for t in range(NT):
                th = thp.tile([P, 2 * KPC], f32, tag="th")
                nc.tensor.matmul(th[:, 0:KPC], sb["fT"][:, t * P:(t + 1) * P],
                                 sb["mT"][:, 0:KPC], start=True, stop=True)
                nc.tensor.matmul(th[:, KPC:], sb["fT"][:, t * P:(t + 1) * P],
                                 sb["mT"][:, KPC:], start=True, stop=True)
                scarg = tp_.tile([P, 2 * KPC], f32, tag="scarg")
                rnd = stp.tile([P, 2 * KPC], f32, tag="rnd")
                nc.vector.tensor_scalar(rnd[:], th[:], RC, RC,
                                        OP.add, OP.subtract)
                nc.vector.tensor_tensor(scarg[:], th[:], rnd[:], OP.subtract)
                scargs.append(scarg)

            # ============ real part, phase 1: invg + d2 + 1/r ============
            # invg[i,j] = 1/sqrt(2(si^2+sj^2)) from a K=2 outer-sum matmul;
            # the true 1/(2*sigma_i) diagonal makes the eps-regularised
            # (i==i, s=0) pair reproduce the analytic self-energy term
            # exactly, so no separate self part is needed.
            sqrt_instrs = []
            for u in range(upc):
                ig = scp.tile([P, P], f32, tag="sc")
                nc.tensor.matmul(ig[:], sb["s2l"][:, u * P:(u + 1) * P],
                                 sb["s2r"][:, u * P:(u + 1) * P],
                                 start=True, stop=True)
                si = nc.scalar.activation(invg_sb[:, u * P:(u + 1) * P],
                                          ig[:], AF.Abs_reciprocal_sqrt,
                                          bias=0.0, scale=2.0)
                if sqrt_instrs:
                    add_dep_helper(si.ins, sqrt_instrs[-1].ins, sync=False,
                                   reason="sqrt unit order")
                sqrt_instrs.append(si)
            # -2b rows built on device from per-unit base + shift (broadcast
            # add), |b|^2 as 0.25 * colsum((-2b)^2) via a K=3 ones matmul;
            # then per 384-chunk: K=3 Gram (-2 a_i . b) + K=1 (+|a_i|^2)
            # + K=1 (+|b|^2) accumulated in psum, drained by one strided
            # Abs_reciprocal_sqrt with a pure-eps bias (the eps must be
            # added after the full cancellation or fp32 rounding eats it)
            for u in range(upc):
                rhs3 = rbp.tile([3, UCOL], f32, tag="rhs3")
                nc.vector.tensor_tensor(
                    rhs3[:].rearrange("p (s j) -> p s j", s=SG),
                    sb["rbase"][:, u * P:(u + 1) * P].unsqueeze(1)
                        .broadcast_to([3, SG, P]),
                    sb["rshift"][:, u * SG:(u + 1) * SG].unsqueeze(2)
                        .broadcast_to([3, SG, P]),
                    OP.add)
                sq3 = rbp.tile([3, UCOL], f32, tag="sq3")
                nc.vector.tensor_tensor(sq3[:], rhs3[:], rhs3[:], OP.mult)
                b2ps = bigp.tile([1, NCH * CHS], f32, tag="big")
                for ch in range(NCH):
                    nc.tensor.matmul(b2ps[0:1, ch * CHS:ch * CHS + CH],
                                     ones3[:], sq3[:, ch * CH:(ch + 1) * CH],
                                     start=True, stop=True)
                b2row = rbp.tile([1, UCOL], f32, tag="b2r")
                nc.vector.tensor_scalar_mul(
                    b2row[:].rearrange("p (c f) -> p c f", c=NCH),
                    b2ps[:].rearrange("p (c f) -> p c f", c=NCH)[:, :, 0:CH],
                    0.25)
                d2 = bigp.tile([P, NCH * CHS], f32, tag="big")
                for ch in range(NCH):
                    nc.tensor.matmul(d2[:, ch * CHS:ch * CHS + CH],
                                     sb["lhsu"][:, u * P:(u + 1) * P],
                                     rhs3[:, ch * CH:(ch + 1) * CH],
                                     start=True, stop=False)
                    nc.tensor.matmul(d2[:, ch * CHS:ch * CHS + CH],
                                     sb["a2r"][0:1, u * P:(u + 1) * P],
                                     onesC[:], start=False, stop=False)
                    nc.tensor.matmul(d2[:, ch * CHS:ch * CHS + CH],
                                     onesP[:],
                                     b2row[0:1, ch * CH:(ch + 1) * CH],
                                     start=False, stop=True)
                si = nc.scalar.activation(
                    rall[:, u * UCOL:(u + 1) * UCOL]
                        .rearrange("p (c f) -> p c f", c=NCH),
                    d2[:].rearrange("p (c f) -> p c f", c=NCH)[:, :, 0:CH],
                    AF.Abs_reciprocal_sqrt, bias=beps[:], scale=1.0)
                add_dep_helper(si.ins, sqrt_instrs[-1].ins, sync=False,
                               reason="sqrt unit order")
                sqrt_instrs.append(si)

            # ---- ACT barrier 1: abs_rsqrt -> erf ----
            bar1t = cp.tile([1, 1], f32)
            b1 = nc.scalar.copy(bar1t[:], ones_t[0:1, :])
            for s in sqrt_instrs:
                add_dep_helper(b1.ins, s.ins, sync=False, reason="act sqrt->erf")

            # ============ real part, phase 2: erf pipeline (fp16) ============
            erf_instrs = []
            for u in range(upc):
                rinv_u = rall[:, u * UCOL:(u + 1) * UCOL]
                H = UCOL // 2
                r_u = wp.tile([P, UCOL], f16, tag="r")
                with nc.allow_low_precision(reason="fp16 erf pipeline"):
                    nc.vector.reciprocal(r_u[:, 0:H], rinv_u[:, 0:H])
                    nc.vector.reciprocal(r_u[:, H:], rinv_u[:, H:])
                erf1 = wp.tile([P, UCOL], f16, tag="erf1")
                for hs in (slice(0, H), slice(H, UCOL)):
                    e1 = nc.scalar.activation(erf1[:, hs], r_u[:, hs], AF.Erf,
                                              bias=0.0, scale=float(c1))
                    add_dep_helper(e1.ins, b1.ins, sync=False,
                                   reason="act sqrt->erf")
                    erf_instrs.append(e1)
                arg2 = wp.tile([P, UCOL], f16, tag="arg2")
                # columns are s-major (col = s*128 + j): broadcast invg over s
                # with the unit-stride j innermost so DVE 2x mode applies
                invg_b = invg_sb[:, u * P:(u + 1) * P].unsqueeze(1) \
                    .broadcast_to([P, SG, P])
                nc.vector.tensor_tensor(
                    arg2[:].rearrange("p (s j) -> p s j", s=SG),
                    r_u[:].rearrange("p (s j) -> p s j", s=SG),
                    invg_b, OP.mult)
                erf2 = wp.tile([P, UCOL], f16, tag="erf2")
                e2 = nc.scalar.activation(erf2[:], arg2[:], AF.Erf,
                                          bias=0.0, scale=1.0)
                add_dep_helper(e2.ins, b1.ins, sync=False, reason="act sqrt->erf")
                erf_instrs.append(e2)
                # diff and valr in place (erf2 <- erf2-erf1 on GPSIMD,
                # erf1 <- diff*rinv on DVE)
                nc.gpsimd.tensor_tensor(erf2[:, 0:H], erf2[:, 0:H],
                                        erf1[:, 0:H], OP.subtract)
                nc.vector.tensor_tensor(erf2[:, H:], erf2[:, H:],
                                        erf1[:, H:], OP.subtract)
                nc.vector.tensor_tensor(erf1[:], erf2[:], rinv_u, OP.mult)
                ast = stp.tile([1, UCOL], f32, tag="ast")
                red = bigp.tile([1, NCH * CHS], f32, tag="big")
                for ch in range(NCH):
                    nc.tensor.matmul(red[0:1, ch * CHS:ch * CHS + CH],
                                     sb["q2c"][:, u:u + 1],
                                     erf1[:, ch * CH:(ch + 1) * CH],
                                     start=True, stop=True)
                nc.vector.tensor_copy(
                    ast[:].rearrange("p (c f) -> p c f", c=NCH),
                    red[:].rearrange("p (c f) -> p c f", c=NCH)[:, :, 0:CH])
                nc.sync.dma_start(arows[u:u + 1, :], ast[:])

            racc_r = cp.tile([upc, 1], f32)
            trash_r = cp.tile([upc, UCOL], f32)
            nc.vector.tensor_tensor(
                trash_r[:].rearrange("u (s j) -> u s j", s=SG),
                arows[:].rearrange("u (s j) -> u s j", s=SG),
                sb["qjr"][:].unsqueeze(1).broadcast_to([upc, SG, P]),
                OP.mult)
            nc.vector.tensor_reduce(racc_r[:], trash_r[:],
                                    axis=mybir.AxisListType.X, op=OP.add)

            # ---- ACT barrier 2: erf -> sin ----
            bar2t = cp.tile([1, 1], f32)
            b2 = nc.scalar.copy(bar2t[:], ones_t[0:1, :])
            for e in erf_instrs:
                add_dep_helper(b2.ins, e.ins, sync=False, reason="act erf->sin")

            # ============ reciprocal part ============
            # psum row: [Sc | Ss] accumulators in one bank
            scrow = scp.tile([1, 2 * KPC], f32, tag="sc")
            sncs = []
            for t in range(NT):
                snc = tp_.tile([P, 2 * KPC], f16, tag="snc")
                si = nc.scalar.activation(snc[:], scargs[t][:], AF.Sin,
                                          bias=0.0, scale=float(2 * math.pi))
                add_dep_helper(si.ins, b2.ins, sync=False, reason="act erf->sin")
                sncs.append(snc)
            for t in range(NT):
                nc.tensor.matmul(scrow[0:1, 0:KPC], sb["qmat16"][:, t:t + 1],
                                 sncs[t][:, KPC:],
                                 start=(t == 0), stop=(t == NT - 1))
            for t in range(NT):
                nc.tensor.matmul(scrow[0:1, KPC:], sb["qmat16"][:, t:t + 1],
                                 sncs[t][:, 0:KPC],
                                 start=(t == 0), stop=(t == NT - 1))
            nc.scalar.copy(scs_st[:], scrow[:])
            nc.sync.dma_start(scs[:], scs_st[:])

            sqk = cp.tile([2, KPC], f32)
            nc.vector.tensor_tensor(sqk[:], scs[:], scs[:], OP.mult)
            racc_k = cp.tile([2, 1], f32)
            trash_k = cp.tile([2, KPC], f32)
            nc.vector.tensor_tensor(trash_k[:], sqk[:], sb["w2"][:], OP.mult)
            nc.vector.tensor_reduce(racc_k[:], trash_k[:],
                                    axis=mybir.AxisListType.X, op=OP.add)

            # ============ combine ============
            # (no separate self part: the eps-regularised diagonal pairs of
            # the real part reproduce it exactly)
            sacc = cp.tile([P, 1], f32)
            nc.vector.memset(sacc[:], 0.0)
            nc.vector.tensor_tensor(sacc[0:upc, :], sacc[0:upc, :],
                                    racc_r[:], OP.add)
            nc.vector.tensor_tensor(sacc[0:2, :], sacc[0:2, :],
                                    racc_k[:], OP.add)
            fin = thp.tile([1, 1], f32, tag="th")
            nc.tensor.matmul(fin[:], sacc[:], ones_t[:], start=True, stop=True)
            outt = cp.tile([1, 1], f32)
            nc.vector.tensor_copy(outt[:], fin[:])
            nc.sync.dma_start(out_d[:], outt[:])
            if dbg:
                nc.sync.dma_start(dbg_d["dbg_invg"][:], invg_sb[:])
                nc.sync.dma_start(dbg_d["dbg_rall"][:], rall[:])
                nc.sync.dma_start(dbg_d["dbg_arows"][:], arows[:])
                nc.sync.dma_start(dbg_d["dbg_rr"][:], racc_r[:])
                nc.sync.dma_start(dbg_d["dbg_rk"][:], racc_k[:])
                nc.sync.dma_start(dbg_d["dbg_scs"][:], scs[:])

    nc.compile()
    _nc_cache[key] = nc
    return nc


def _shift_grid(n):
    r = np.arange(-n, n + 1, dtype=np.float64)
    g = np.stack(np.meshgrid(r, r, r, indexing="ij"), axis=-1)
    return g.reshape(-1, 3)


def prep_in_maps(pos, cell, charges, sigma_table, species_idx):
    """Host-side shard prep: returns (in_maps list of 8 dicts, c1, upc)."""
    pos = np.asarray(pos, np.float32)
    cell = np.asarray(cell, np.float32)
    if cell.ndim == 3:
        cell = cell[0]
    q = np.asarray(charges, np.float32).reshape(-1)
    sigma_table = np.asarray(sigma_table, np.float32)
    species_idx = np.asarray(species_idx).astype(np.int64)
    sigmas = sigma_table[species_idx]

    vol = abs(np.linalg.det(cell.astype(np.float64)))
    eta = (vol ** 2 / N) ** (1.0 / 6.0) / math.sqrt(2.0 * math.pi)
    cutoff_recip = math.sqrt(-2.0 * math.log(1e-8)) / eta
    cutoff_real = math.sqrt(-2.0 * math.log(1e-8)) * eta
    c1 = 1.0 / (math.sqrt(2.0) * eta)

    # sort atoms along x so the 128-atom tiles become x-slabs (enables exact
    # culling of far tile-pair/shift units)
    perm = np.argsort(pos[:, 0], kind="stable")
    pos = pos[perm]
    q = q[perm]
    sigmas = sigmas[perm]

    center = 0.5 * cell.astype(np.float64).sum(axis=0)
    a = (pos.astype(np.float64) - center).astype(np.float32)
    a2 = (a * a).sum(1).astype(np.float32)
    shifts = (_shift_grid(NSHIFT_REAL) @ cell.astype(np.float64)).astype(np.float32)

    sig2 = sigmas.astype(np.float32) ** 2

    units = _select_units(a, shifts, cutoff_real)
    upc = max(1, (len(units) + NCORES - 1) // NCORES)
    units = units + [None] * (NCORES * upc - len(units))

    # reciprocal k-grid: keep only k with nonzero weight (exact culling)
    gk = _shift_grid(NSHIFT_RECIP)                     # (4913, 3) float64
    recip = 2.0 * math.pi * np.linalg.inv(cell.astype(np.float64)).T
    ks_all = gk @ recip
    klen_all = np.linalg.norm(ks_all, axis=-1)
    kmask = (klen_all > 1e-8) & (klen_all < cutoff_recip)
    kidx = np.nonzero(kmask)[0]
    KTOT = NCORES * KPC
    assert len(kidx) <= KTOT, f"{len(kidx)} active k > {KTOT} slots"
    gk_pad = np.zeros((KTOT, 3), np.float64)
    gk_pad[: len(kidx)] = gk[kidx]
    wk = np.zeros(KTOT, np.float64)
    wk[: len(kidx)] = (np.exp(-0.5 * (eta * klen_all[kidx]) ** 2)
                       / klen_all[kidx] ** 2)
    wk = wk * (0.5 * COEF * 4.0 * math.pi / vol)
    frac = pos.astype(np.float64) @ np.linalg.inv(cell.astype(np.float64))
    fT_all = frac.T.astype(np.float32)                 # (3, N)

    # no separate self part: with invg_ii = 1/(2 sigma_i) the device's
    # eps-regularised (i==i, s=0) pairs give (erf(r invg) - erf(r c1))/r
    # -> (2/sqrt(pi))(1/(2 sigma_i) - 1/(sqrt2 eta))
    #  = 1/(sqrt(pi) sigma_i) - sqrt(2/pi)/eta, the analytic self term.

    in_maps = []
    for c in range(NCORES):
        lhsu = np.zeros((3, upc * P), np.float32)
        a2r = np.zeros((1, upc * P), np.float32)
        rbase = np.zeros((3, upc * P), np.float32)
        rshift = np.zeros((3, upc * SG), np.float32)
        s2l = np.ones((2, upc * P), np.float32)
        s2r = np.ones((2, upc * P), np.float32)
        q2c = np.zeros((P, upc), np.float32)
        qjr = np.zeros((upc, P), np.float32)
        for k in range(upc):
            unit = units[c * upc + k]
            if unit is None:
                continue   # zero-weight dummy; s2l/s2r stay 1 -> finite invg
            ti, tj, sg = unit
            wu = 1.0 if ti == tj else 2.0
            ai = a[ti * P:(ti + 1) * P]                # (128, 3)
            lhsu[:, k * P:(k + 1) * P] = ai.T
            a2r[0, k * P:(k + 1) * P] = a2[ti * P:(ti + 1) * P]
            aj = a[tj * P:(tj + 1) * P]                # (128, 3)
            # device builds -2b[s,j] = -2 a_j + -2 shift_s (s-major columns)
            rbase[:, k * P:(k + 1) * P] = -2.0 * aj.T
            rshift[:, k * SG:(k + 1) * SG] = \
                -2.0 * shifts[sg * SG:(sg + 1) * SG].T
            s2l[0, k * P:(k + 1) * P] = sig2[ti * P:(ti + 1) * P]
            s2r[1, k * P:(k + 1) * P] = sig2[tj * P:(tj + 1) * P]
            q2c[:, k] = q[ti * P:(ti + 1) * P] * np.float32(0.5 * COEF * wu)
            qjr[k] = q[tj * P:(tj + 1) * P]
        ksl = slice(c * KPC, (c + 1) * KPC)
        mTc = np.empty((3, 2 * KPC), np.float32)
        mTc[:, 0:KPC] = gk_pad[ksl].T.astype(np.float32)
        mTc[:, KPC:] = mTc[:, 0:KPC]
        w2c = np.broadcast_to(wk[ksl].astype(np.float32), (2, KPC)).copy()
        in_maps.append({
            "lhsu": lhsu, "a2r": a2r, "rbase": rbase, "rshift": rshift,
            "s2l": s2l, "s2r": s2r,
            "q2c": q2c.astype(np.float16), "qjr": qjr,
            "mT": mTc, "fT": fT_all.copy(), "w2": w2c,
            "qmat16": q.reshape(NT, P).T.astype(np.float16),
        })
    return in_maps, c1, upc


_runner_cache = {}


def _make_runner(nc, n_cores):
    """Build the jitted SPMD executable for ``nc`` ONCE.

    ``bass_utils.run_bass_kernel_spmd`` (axon path) creates a fresh closure
    and a fresh ``jax.jit`` on every call, so every invocation re-traces,
    re-lowers and re-runs ``compile_bir_kernel``/DVE-table generation
    (~200 ms of host work per call).  This mirrors its exact execution
    semantics (same ``_bass_exec_p`` bind, same shard_map layout, same
    donated zero-initialised outputs) but hoists all of that out of the
    per-call path: steady-state calls are just concat + dispatch + fetch.
    """
    import jax
    from concourse import bass2jax, mybir

    bass2jax.install_neuronx_cc_hook()
    if nc.dbg_addr is not None and nc.dbg_callbacks:
        raise RuntimeError("dbg callbacks unsupported in cached runner")
    partition_name = nc.partition_id_tensor.name if nc.partition_id_tensor else None
    dbg_name = nc.dbg_addr.name if nc.dbg_addr is not None else None

    in_names, in_specs_np, out_names, out_avals = [], [], [], []
    for alloc in nc.m.functions[0].allocations:
        if not isinstance(alloc, mybir.MemoryLocationSet):
            continue
        name = alloc.memorylocations[0].name
        if alloc.kind == "ExternalInput":
            if name != partition_name:
                in_names.append(name)
                in_specs_np.append((tuple(alloc.tensor_shape),
                                    mybir.dt.np(alloc.dtype)))
        elif alloc.kind == "ExternalOutput":
            out_names.append(name)
            out_avals.append(jax.core.ShapedArray(
                tuple(alloc.tensor_shape), mybir.dt.np(alloc.dtype)))
    n_params = len(in_names)
    n_outs = len(out_names)
    all_names = in_names + out_names + ([partition_name] if partition_name else [])
    donate = tuple(range(n_params, n_params + n_outs))

    def _body(*args):
        operands = list(args)
        if partition_name is not None:
            operands.append(bass2jax.partition_id_tensor())
        return tuple(bass2jax._bass_exec_p.bind(
            *operands, out_avals=tuple(out_avals), in_names=tuple(all_names),
            out_names=tuple(out_names), lowering_input_output_aliases=(),
            sim_require_finite=True, sim_require_nnan=True, nc=nc))

    devices = jax.devices()[:n_cores]
    assert len(devices) == n_cores
    mesh = bass2jax.Mesh(np.asarray(devices), ("core",))
    PS = bass2jax.PartitionSpec
    sharded = jax.jit(
        bass2jax.shard_map(_body, mesh=mesh,
                           in_specs=(PS("core"),) * (n_params + n_outs),
                           out_specs=(PS("core"),) * n_outs,
                           check_rep=False),
        donate_argnums=donate, keep_unused=True)
    # AOT-compile to skip the python pjit dispatch path (~3-8 ms/call)
    try:
        structs = [jax.ShapeDtypeStruct((n_cores * s[0], *s[1:]), dt)
                   for s, dt in in_specs_np]
        structs += [jax.ShapeDtypeStruct((n_cores * av.shape[0],
                                          *av.shape[1:]), av.dtype)
                    for av in out_avals]
        call = sharded.lower(*structs).compile()
    except Exception:
        call = sharded

    def run(in_maps):
        if dbg_name is not None:
            in_maps = [{**m, dbg_name: np.zeros((1, 2), np.uint32)}
                       for m in in_maps]
        concat_in = [
            np.concatenate([np.asarray(m[name]) for m in in_maps], axis=0)
            for name in in_names]
        concat_zeros = [
            np.zeros((n_cores * av.shape[0], *av.shape[1:]), av.dtype)
            for av in out_avals]
        out_arrs = call(*concat_in, *concat_zeros)
        outs_np = [np.asarray(o) for o in out_arrs]
        return [{name: outs_np[i].reshape(n_cores, *out_avals[i].shape)[c]
                 for i, name in enumerate(out_names)}
                for c in range(n_cores)]

    return run


def _run_spmd(nc, in_maps):
    """Run ``nc`` on 8 cores; cached-jit fast path with library fallback."""
    key = id(nc)
    run = _runner_cache.get(key)
    if run is None:
        try:
            run = _make_runner(nc, NCORES)
            _runner_cache[key] = run
        except Exception:
            run = None
    if run is not None:
        try:
            return run(in_maps)
        except Exception:
            _runner_cache.pop(key, None)
    from concourse import bass_utils
    res = bass_utils.run_bass_kernel_spmd(nc, in_maps,
                                          core_ids=list(range(NCORES)))
    return res.results


def kernel(pos, cell, charges, sigma_table, species_idx,
           nshift_real, nshift_recip):
    assert int(nshift_real) == NSHIFT_REAL and int(nshift_recip) == NSHIFT_RECIP, \
        "kernel compiled for nshift_real=1, nshift_recip=8"
    pos = np.asarray(pos)
    assert pos.shape == (N, 3)

    in_maps, c1, upc = prep_in_maps(pos, cell, charges, sigma_table,
                                    species_idx)
    nc = build_program(c1, upc)

    results = _run_spmd(nc, in_maps)
    e = np.float64(0.0)
    for i in range(NCORES):
        e += np.float64(results[i]["out"][0, 0])
    return np.array([[e]], dtype=np.float32)



# revision 52
# speedup vs baseline: 1.1728x; 1.1686x over previous
"""Trainium2 Bass kernel for the Ewald energy nn.Module.

Math restructuring (validated to ~2.6e-4 rel err against the jax reference
with the fp16 fast path):
  E = E_real + E_recip with the charge contraction folded in:
    E_real  = 0.5*COEF * sum_s sum_ij q_i q_j (erf(r/(sqrt2*gam_ij)) - erf(r/(sqrt2*eta)))/r
              over the symmetric half of the 6x6 grid of 128-atom tile pairs
              (off-diagonal pairs weighted 2x).  d^2 comes from Gram-matrix
              matmuls on the TensorEngine; atoms are pre-sorted along x so
              (tile-pair, x-shift-group) units whose x-gap exceeds the
              real-space cutoff are culled exactly (the reference masks
              r>cutoff).  With the true invgamma[i,i] = 1/(2 sigma_i), the
              eps-regularised (i==i, s=0) pairs reproduce the analytic
              self-energy term exactly, so no separate self part exists.
    E_recip = 0.5*COEF*(4pi/V) * sum_k w_k * (Sc_k^2 + Ss_k^2), structure
              factors via PE matmuls; only the ~1646 nonzero-weight k of the
              17^3 grid are computed.  theta is range-reduced via fractional
              coordinates and the (y + 1.5*2^23) - 1.5*2^23 round trick so
              Sin stays inside its [-pi, pi] table range.

Sharding: surviving real-space units are distributed round-robin over the 8
cores (UPC slots each, dummies zero-weighted); active k split 256 per core;
every core returns a scalar partial and the host sums the 8.

The run path is latency-dominated (axon tunnel RTT ~35 ms, measured
wall-clock), so the kernel minimises per-call host work and bytes shipped:
  * ~50 KB/core of inputs.  Redundant tensors are rebuilt on device: the
    9-shift -2b rows from a per-unit base row + shift constants (broadcast
    add), |b|^2 = 0.25*colsum((-2b)^2) via a K=3 ones matmul, invgamma from
    sigma^2 vectors via a K=2 outer-sum matmul + Abs_reciprocal_sqrt, q_j
    broadcast over shifts in the final reduce, and the constant rows of the
    theta operands (memset row 0 - compute-engine partition ranges must
    start at 0/32/64).
  * d^2 accumulates K=3 (-2 a_i . b) + K=1 (+|a_i|^2) + K=1 (+|b|^2)
    matmuls in psum; the 1e-8 floor rides the activation bias so it is
    added AFTER the Gram cancellation (folding it into |a_i|^2 loses it to
    fp32 rounding and yields 1/sqrt(0) = inf on the diagonal).
  * _run_spmd caches an AOT-compiled jitted shard_map executable per
    program (the library helper re-traces, re-lowers and re-runs DVE-table
    generation on EVERY call, ~200 ms of host work).

ScalarEngine activations are phase-ordered (Abs_reciprocal_sqrt -> Erf ->
Sin) via emission order plus explicit scheduler edges through two tiny ACT
barrier copies, so each ACT table set loads exactly once.  The erf pipeline
runs in fp16 to unlock the DVE 2x perf mode.
"""
import math
import os
import sys
import numpy as np

_TRN_REPO = "/opt/trn_rl_repo"
if _TRN_REPO not in sys.path and os.path.isdir(_TRN_REPO):
    sys.path.insert(0, _TRN_REPO)

COEF = 14.399645478425668
N = 768
NT = 6            # 128-atom tiles
P = 128
NSHIFT_REAL = 1   # -> 27 shifts, 3 x-groups of 9
NSG = 3
SG = 9
UCOL = P * SG     # 1152 columns per unit
CH = 384          # matmul chunk
CHS = 512         # psum chunk stride (bank aligned)
NCH = 3
NCORES = 8
NSHIFT_RECIP = 8  # -> 17^3 = 4913 k-vectors; ~1646 carry weight
KPC = 256         # active k per core
_MAX_UNITS = 48   # provable upper bound on surviving units

_nc_cache = {}


def _PACK32(upc):
    """(name, rows, cols) layout of the flat f32 input buffer."""
    return [
        ("lhsu", 3, upc * P),          # a_i rows per unit
        ("a2r", 1, upc * P),           # |a_i|^2 per unit (single row)
        ("rbase", 3, upc * P),         # -2*a_j rows per unit
        ("rshift", 3, upc * SG),       # -2*shift rows per unit
        ("s2l", 2, upc * P),           # [sigma_i^2; 1] per unit
        ("s2r", 2, upc * P),           # [1; sigma_j^2] per unit
        ("qjr", upc, P),               # q_j per unit (broadcast over shifts)
        ("mT", 3, 2 * KPC),            # k rows; 24.0/24.25 row on device
        ("fT", 3, N),                  # frac rows; ones row on device
        ("w2", 2, KPC),
    ]


def _PACK16(upc):
    """(name, rows, cols) layout of the flat f16 input buffer."""
    return [
        ("q2c", P, upc),
        ("qmat16", P, NT),
    ]


def _tile_pairs():
    return [(ti, tj) for ti in range(NT) for tj in range(ti, NT)]


def _select_units(a, shifts, cutoff):
    """Cull (tilepair, shift-group) units whose x-gap exceeds the real-space
    cutoff.  Requires atoms sorted by x; sorted blocks guarantee at most 48
    survivors."""
    x = a[:, 0]
    lo = [x[t * P:(t + 1) * P].min() for t in range(NT)]
    hi = [x[t * P:(t + 1) * P].max() for t in range(NT)]
    units = []
    for (ti, tj) in _tile_pairs():
        for sg in range(NSG):
            keep = False
            for s in range(SG):
                sx = float(shifts[sg * SG + s, 0])
                d_lo = lo[tj] + sx - hi[ti]
                d_hi = hi[tj] + sx - lo[ti]
                if not (d_lo > cutoff or d_hi < -cutoff):
                    keep = True
                    break
            if keep:
                units.append((ti, tj, sg))
    assert len(units) <= _MAX_UNITS, f"{len(units)} units > {_MAX_UNITS}"
    return units


def build_program(c1, upc, dbg=False):
    """Build + compile the per-core Bass program (same on all cores).

    c1 = 1/(sqrt(2)*eta) is baked in as the erf scale constant; upc is the
    number of real-space unit slots per core.
    """
    key = ("nc", round(float(c1), 12), int(upc), bool(dbg))
    if key in _nc_cache:
        return _nc_cache[key]

    import concourse.bacc as bacc
    import concourse.tile as tile
    from concourse import mybir
    from concourse.tile import add_dep_helper

    AF = mybir.ActivationFunctionType
    OP = mybir.AluOpType
    f32 = mybir.dt.float32
    f16 = mybir.dt.float16

    nc = bacc.Bacc("TRN2", target_bir_lowering=False, debug=False)

    dt_in = {}

    def din(name, shape, dtype=f32):
        dt_in[name] = nc.dram_tensor(name, shape, dtype,
                                     kind="ExternalInput").ap()
        return dt_in[name]

    # all per-core tensors ride in two flat buffers (one per dtype) to cut
    # per-array tunnel overhead; device DMAs the slices into 2-D tiles
    specs32 = _PACK32(upc)
    specs16 = _PACK16(upc)
    n32 = sum(r * c for _, r, c in specs32)
    n16 = sum(r * c for _, r, c in specs16)
    din("pk32", [1, n32])
    din("pk16", [1, n16], f16)
    out_d = nc.dram_tensor("out", [1, 1], f32, kind="ExternalOutput").ap()
    if dbg:
        dbg_d = {
            "dbg_invg": nc.dram_tensor("dbg_invg", [P, upc * P], f16,
                                       kind="ExternalOutput").ap(),
            "dbg_rall": nc.dram_tensor("dbg_rall", [P, upc * UCOL], f16,
                                       kind="ExternalOutput").ap(),
            "dbg_arows": nc.dram_tensor("dbg_arows", [upc, UCOL], f32,
                                        kind="ExternalOutput").ap(),
            "dbg_rr": nc.dram_tensor("dbg_rr", [upc, 1], f32,
                                     kind="ExternalOutput").ap(),
            "dbg_rk": nc.dram_tensor("dbg_rk", [2, 1], f32,
                                     kind="ExternalOutput").ap(),
            "dbg_scs": nc.dram_tensor("dbg_scs", [2, KPC], f32,
                                      kind="ExternalOutput").ap(),
        }

    with tile.TileContext(nc) as tc:
        with tc.tile_pool(name="consts", bufs=1) as cp, \
             tc.tile_pool(name="rall", bufs=1) as rp, \
             tc.tile_pool(name="trig", bufs=6) as tp_, \
             tc.tile_pool(name="work", bufs=2) as wp, \
             tc.tile_pool(name="rbuild", bufs=2) as rbp, \
             tc.tile_pool(name="stage", bufs=2) as stp, \
             tc.tile_pool(name="scps", bufs=1, space="PSUM") as scp, \
             tc.tile_pool(name="bigps", bufs=2, space="PSUM") as bigp, \
             tc.tile_pool(name="thps", bufs=1, space="PSUM") as thp:

            # ---- unpack inputs to SBUF ----
            # fT/mT get a constant row 0 prepended on device (memset must
            # start at partition 0; DMA lands the data on partitions 1-3)
            sb = {}
            for specs, buf, dt in ((specs32, "pk32", f32),
                                   (specs16, "pk16", f16)):
                off = 0
                for name, r, c in specs:
                    if name in ("fT", "mT"):
                        t = cp.tile([4, c], dt, name=f"sb_{name}")
                        dst = t[1:4, :]
                    else:
                        t = cp.tile([r, c], dt, name=f"sb_{name}")
                        dst = t[:]
                    src = dt_in[buf][0:1, off:off + r * c] \
                        .rearrange("a (p f) -> (a p) f", f=c)
                    nc.sync.dma_start(dst, src)
                    sb[name] = t
                    off += r * c
            nc.vector.memset(sb["fT"][0:1, :], 1.0)
            nc.vector.memset(sb["mT"][0:1, 0:KPC], 24.0)
            nc.vector.memset(sb["mT"][0:1, KPC:], 24.25)

            beps = cp.tile([P, 1], f32)
            nc.vector.memset(beps[:], 1e-8)
            ones_t = cp.tile([P, 1], f32)
            nc.vector.memset(ones_t[:], 1.0)
            onesP = cp.tile([1, P], f32)           # K=1 lhs for the b^2 matmul
            nc.vector.memset(onesP[:], 1.0)
            onesC = cp.tile([1, CH], f32)          # K=1 rhs for the a^2 matmul
            nc.vector.memset(onesC[:], 1.0)
            ones3 = cp.tile([3, 1], f32)           # K=3 lhs for the b^2 colsum
            nc.vector.memset(ones3[:], 1.0)

            rall = rp.tile([P, upc * UCOL], f16)   # 1/r for all units
            invg_sb = cp.tile([P, upc * P], f16)   # 1/sqrt(2(si^2+sj^2))
            arows = cp.tile([upc, UCOL], f32)      # per-unit reduced rows
            scs = cp.tile([2, KPC], f32)           # [Sc; Ss] (squared later)
            scs_st = cp.tile([1, 2 * KPC], f32)    # partition-0 staging

            # ============ recip theta + range reduction (early) ============
            scargs = []
            RC = 12582912.0  # 1.5 * 2**23: (y + RC) - RC == round-to-nearest
            for t in range(NT):
                th = thp.tile([P, 2 * KPC], f32, tag="th")
                nc.tensor.matmul(th[:, 0:KPC], sb["fT"][:, t * P:(t + 1) * P],
                                 sb["mT"][:, 0:KPC], start=True, stop=True)
                nc.tensor.matmul(th[:, KPC:], sb["fT"][:, t * P:(t + 1) * P],
                                 sb["mT"][:, KPC:], start=True, stop=True)
                scarg = tp_.tile([P, 2 * KPC], f32, tag="scarg")
                rnd = stp.tile([P, 2 * KPC], f32, tag="rnd")
                nc.vector.tensor_scalar(rnd[:], th[:], RC, RC,
                                        OP.add, OP.subtract)
                nc.vector.tensor_tensor(scarg[:], th[:], rnd[:], OP.subtract)
                scargs.append(scarg)

            # ============ real part, phase 1: invg + d2 + 1/r ============
            # invg[i,j] = 1/sqrt(2(si^2+sj^2)) from a K=2 outer-sum matmul;
            # the true 1/(2*sigma_i) diagonal makes the eps-regularised
            # (i==i, s=0) pair reproduce the analytic self-energy term
            # exactly, so no separate self part is needed.
            sqrt_instrs = []
            for u in range(upc):
                ig = scp.tile([P, P], f32, tag="sc")
                nc.tensor.matmul(ig[:], sb["s2l"][:, u * P:(u + 1) * P],
                                 sb["s2r"][:, u * P:(u + 1) * P],
                                 start=True, stop=True)
                si = nc.scalar.activation(invg_sb[:, u * P:(u + 1) * P],
                                          ig[:], AF.Abs_reciprocal_sqrt,
                                          bias=0.0, scale=2.0)
                if sqrt_instrs:
                    add_dep_helper(si.ins, sqrt_instrs[-1].ins, sync=False,
                                   reason="sqrt unit order")
                sqrt_instrs.append(si)
            # -2b rows built on device from per-unit base + shift (broadcast
            # add), |b|^2 as 0.25 * colsum((-2b)^2) via a K=3 ones matmul;
            # then per 384-chunk: K=3 Gram (-2 a_i . b) + K=1 (+|a_i|^2)
            # + K=1 (+|b|^2) accumulated in psum, drained by one strided
            # Abs_reciprocal_sqrt with a pure-eps bias (the eps must be
            # added after the full cancellation or fp32 rounding eats it)
            for u in range(upc):
                rhs3 = rbp.tile([3, UCOL], f32, tag="rhs3")
                nc.vector.tensor_tensor(
                    rhs3[:].rearrange("p (s j) -> p s j", s=SG),
                    sb["rbase"][:, u * P:(u + 1) * P].unsqueeze(1)
                        .broadcast_to([3, SG, P]),
                    sb["rshift"][:, u * SG:(u + 1) * SG].unsqueeze(2)
                        .broadcast_to([3, SG, P]),
                    OP.add)
                sq3 = rbp.tile([3, UCOL], f32, tag="sq3")
                nc.vector.tensor_tensor(sq3[:], rhs3[:], rhs3[:], OP.mult)
                b2ps = bigp.tile([1, NCH * CHS], f32, tag="big")
                for ch in range(NCH):
                    nc.tensor.matmul(b2ps[0:1, ch * CHS:ch * CHS + CH],
                                     ones3[:], sq3[:, ch * CH:(ch + 1) * CH],
                                     start=True, stop=True)
                b2row = rbp.tile([1, UCOL], f32, tag="b2r")
                nc.vector.tensor_scalar_mul(
                    b2row[:].rearrange("p (c f) -> p c f", c=NCH),
                    b2ps[:].rearrange("p (c f) -> p c f", c=NCH)[:, :, 0:CH],
                    0.25)
                d2 = bigp.tile([P, NCH * CHS], f32, tag="big")
                for ch in range(NCH):
                    nc.tensor.matmul(d2[:, ch * CHS:ch * CHS + CH],
                                     sb["lhsu"][:, u * P:(u + 1) * P],
                                     rhs3[:, ch * CH:(ch + 1) * CH],
                                     start=True, stop=False)
                    nc.tensor.matmul(d2[:, ch * CHS:ch * CHS + CH],
                                     sb["a2r"][0:1, u * P:(u + 1) * P],
                                     onesC[:], start=False, stop=False)
                    nc.tensor.matmul(d2[:, ch * CHS:ch * CHS + CH],
                                     onesP[:],
                                     b2row[0:1, ch * CH:(ch + 1) * CH],
                                     start=False, stop=True)
                si = nc.scalar.activation(
                    rall[:, u * UCOL:(u + 1) * UCOL]
                        .rearrange("p (c f) -> p c f", c=NCH),
                    d2[:].rearrange("p (c f) -> p c f", c=NCH)[:, :, 0:CH],
                    AF.Abs_reciprocal_sqrt, bias=beps[:], scale=1.0)
                add_dep_helper(si.ins, sqrt_instrs[-1].ins, sync=False,
                               reason="sqrt unit order")
                sqrt_instrs.append(si)

            # ---- ACT barrier 1: abs_rsqrt -> erf ----
            bar1t = cp.tile([1, 1], f32)
            b1 = nc.scalar.copy(bar1t[:], ones_t[0:1, :])
            for s in sqrt_instrs:
                add_dep_helper(b1.ins, s.ins, sync=False, reason="act sqrt->erf")

            # ============ real part, phase 2: erf pipeline (fp16) ============
            erf_instrs = []
            for u in range(upc):
                rinv_u = rall[:, u * UCOL:(u + 1) * UCOL]
                H = UCOL // 2
                r_u = wp.tile([P, UCOL], f16, tag="r")
                with nc.allow_low_precision(reason="fp16 erf pipeline"):
                    nc.vector.reciprocal(r_u[:, 0:H], rinv_u[:, 0:H])
                    nc.vector.reciprocal(r_u[:, H:], rinv_u[:, H:])
                erf1 = wp.tile([P, UCOL], f16, tag="erf1")
                for hs in (slice(0, H), slice(H, UCOL)):
                    e1 = nc.scalar.activation(erf1[:, hs], r_u[:, hs], AF.Erf,
                                              bias=0.0, scale=float(c1))
                    add_dep_helper(e1.ins, b1.ins, sync=False,
                                   reason="act sqrt->erf")
                    erf_instrs.append(e1)
                arg2 = wp.tile([P, UCOL], f16, tag="arg2")
                # columns are s-major (col = s*128 + j): broadcast invg over s
                # with the unit-stride j innermost so DVE 2x mode applies
                invg_b = invg_sb[:, u * P:(u + 1) * P].unsqueeze(1) \
                    .broadcast_to([P, SG, P])
                nc.vector.tensor_tensor(
                    arg2[:].rearrange("p (s j) -> p s j", s=SG),
                    r_u[:].rearrange("p (s j) -> p s j", s=SG),
                    invg_b, OP.mult)
                erf2 = wp.tile([P, UCOL], f16, tag="erf2")
                e2 = nc.scalar.activation(erf2[:], arg2[:], AF.Erf,
                                          bias=0.0, scale=1.0)
                add_dep_helper(e2.ins, b1.ins, sync=False, reason="act sqrt->erf")
                erf_instrs.append(e2)
                # diff and valr in place (erf2 <- erf2-erf1 on GPSIMD,
                # erf1 <- diff*rinv on DVE)
                nc.gpsimd.tensor_tensor(erf2[:, 0:H], erf2[:, 0:H],
                                        erf1[:, 0:H], OP.subtract)
                nc.vector.tensor_tensor(erf2[:, H:], erf2[:, H:],
                                        erf1[:, H:], OP.subtract)
                nc.vector.tensor_tensor(erf1[:], erf2[:], rinv_u, OP.mult)
                ast = stp.tile([1, UCOL], f32, tag="ast")
                red = bigp.tile([1, NCH * CHS], f32, tag="big")
                for ch in range(NCH):
                    nc.tensor.matmul(red[0:1, ch * CHS:ch * CHS + CH],
                                     sb["q2c"][:, u:u + 1],
                                     erf1[:, ch * CH:(ch + 1) * CH],
                                     start=True, stop=True)
                nc.vector.tensor_copy(
                    ast[:].rearrange("p (c f) -> p c f", c=NCH),
                    red[:].rearrange("p (c f) -> p c f", c=NCH)[:, :, 0:CH])
                nc.sync.dma_start(arows[u:u + 1, :], ast[:])

            racc_r = cp.tile([upc, 1], f32)
            trash_r = cp.tile([upc, UCOL], f32)
            nc.vector.tensor_tensor(
                trash_r[:].rearrange("u (s j) -> u s j", s=SG),
                arows[:].rearrange("u (s j) -> u s j", s=SG),
                sb["qjr"][:].unsqueeze(1).broadcast_to([upc, SG, P]),
                OP.mult)
            nc.vector.tensor_reduce(racc_r[:], trash_r[:],
                                    axis=mybir.AxisListType.X, op=OP.add)

            # ---- ACT barrier 2: erf -> sin ----
            bar2t = cp.tile([1, 1], f32)
            b2 = nc.scalar.copy(bar2t[:], ones_t[0:1, :])
            for e in erf_instrs:
                add_dep_helper(b2.ins, e.ins, sync=False, reason="act erf->sin")

            # ============ reciprocal part ============
            # psum row: [Sc | Ss] accumulators in one bank
            scrow = scp.tile([1, 2 * KPC], f32, tag="sc")
            sncs = []
            for t in range(NT):
                snc = tp_.tile([P, 2 * KPC], f16, tag="snc")
                si = nc.scalar.activation(snc[:], scargs[t][:], AF.Sin,
                                          bias=0.0, scale=float(2 * math.pi))
                add_dep_helper(si.ins, b2.ins, sync=False, reason="act erf->sin")
                sncs.append(snc)
            for t in range(NT):
                nc.tensor.matmul(scrow[0:1, 0:KPC], sb["qmat16"][:, t:t + 1],
                                 sncs[t][:, KPC:],
                                 start=(t == 0), stop=(t == NT - 1))
            for t in range(NT):
                nc.tensor.matmul(scrow[0:1, KPC:], sb["qmat16"][:, t:t + 1],
                                 sncs[t][:, 0:KPC],
                                 start=(t == 0), stop=(t == NT - 1))
            nc.scalar.copy(scs_st[:], scrow[:])
            nc.sync.dma_start(scs[:], scs_st[:])

            sqk = cp.tile([2, KPC], f32)
            nc.vector.tensor_tensor(sqk[:], scs[:], scs[:], OP.mult)
            racc_k = cp.tile([2, 1], f32)
            trash_k = cp.tile([2, KPC], f32)
            nc.vector.tensor_tensor(trash_k[:], sqk[:], sb["w2"][:], OP.mult)
            nc.vector.tensor_reduce(racc_k[:], trash_k[:],
                                    axis=mybir.AxisListType.X, op=OP.add)

            # ============ combine ============
            # (no separate self part: the eps-regularised diagonal pairs of
            # the real part reproduce it exactly)
            sacc = cp.tile([P, 1], f32)
            nc.vector.memset(sacc[:], 0.0)
            nc.vector.tensor_tensor(sacc[0:upc, :], sacc[0:upc, :],
                                    racc_r[:], OP.add)
            nc.vector.tensor_tensor(sacc[0:2, :], sacc[0:2, :],
                                    racc_k[:], OP.add)
            fin = thp.tile([1, 1], f32, tag="th")
            nc.tensor.matmul(fin[:], sacc[:], ones_t[:], start=True, stop=True)
            outt = cp.tile([1, 1], f32)
            nc.vector.tensor_copy(outt[:], fin[:])
            nc.sync.dma_start(out_d[:], outt[:])
            if dbg:
                nc.sync.dma_start(dbg_d["dbg_invg"][:], invg_sb[:])
                nc.sync.dma_start(dbg_d["dbg_rall"][:], rall[:])
                nc.sync.dma_start(dbg_d["dbg_arows"][:], arows[:])
                nc.sync.dma_start(dbg_d["dbg_rr"][:], racc_r[:])
                nc.sync.dma_start(dbg_d["dbg_rk"][:], racc_k[:])
                nc.sync.dma_start(dbg_d["dbg_scs"][:], scs[:])

    nc.compile()
    _nc_cache[key] = nc
    return nc


def _shift_grid(n):
    r = np.arange(-n, n + 1, dtype=np.float64)
    g = np.stack(np.meshgrid(r, r, r, indexing="ij"), axis=-1)
    return g.reshape(-1, 3)


def prep_in_maps(pos, cell, charges, sigma_table, species_idx):
    """Host-side shard prep: returns (in_maps list of 8 dicts, c1, upc)."""
    pos = np.asarray(pos, np.float32)
    cell = np.asarray(cell, np.float32)
    if cell.ndim == 3:
        cell = cell[0]
    q = np.asarray(charges, np.float32).reshape(-1)
    sigma_table = np.asarray(sigma_table, np.float32)
    species_idx = np.asarray(species_idx).astype(np.int64)
    sigmas = sigma_table[species_idx]

    vol = abs(np.linalg.det(cell.astype(np.float64)))
    eta = (vol ** 2 / N) ** (1.0 / 6.0) / math.sqrt(2.0 * math.pi)
    cutoff_recip = math.sqrt(-2.0 * math.log(1e-8)) / eta
    cutoff_real = math.sqrt(-2.0 * math.log(1e-8)) * eta
    c1 = 1.0 / (math.sqrt(2.0) * eta)

    # sort atoms along x so the 128-atom tiles become x-slabs (enables exact
    # culling of far tile-pair/shift units)
    perm = np.argsort(pos[:, 0], kind="stable")
    pos = pos[perm]
    q = q[perm]
    sigmas = sigmas[perm]

    center = 0.5 * cell.astype(np.float64).sum(axis=0)
    a = (pos.astype(np.float64) - center).astype(np.float32)
    a2 = (a * a).sum(1).astype(np.float32)
    shifts = (_shift_grid(NSHIFT_REAL) @ cell.astype(np.float64)).astype(np.float32)

    sig2 = sigmas.astype(np.float32) ** 2

    units = _select_units(a, shifts, cutoff_real)
    upc = max(1, (len(units) + NCORES - 1) // NCORES)
    units = units + [None] * (NCORES * upc - len(units))

    # reciprocal k-grid: keep only k with nonzero weight (exact culling)
    gk = _shift_grid(NSHIFT_RECIP)                     # (4913, 3) float64
    recip = 2.0 * math.pi * np.linalg.inv(cell.astype(np.float64)).T
    ks_all = gk @ recip
    klen_all = np.linalg.norm(ks_all, axis=-1)
    kmask = (klen_all > 1e-8) & (klen_all < cutoff_recip)
    kidx = np.nonzero(kmask)[0]
    KTOT = NCORES * KPC
    assert len(kidx) <= KTOT, f"{len(kidx)} active k > {KTOT} slots"
    gk_pad = np.zeros((KTOT, 3), np.float64)
    gk_pad[: len(kidx)] = gk[kidx]
    wk = np.zeros(KTOT, np.float64)
    wk[: len(kidx)] = (np.exp(-0.5 * (eta * klen_all[kidx]) ** 2)
                       / klen_all[kidx] ** 2)
    wk = wk * (0.5 * COEF * 4.0 * math.pi / vol)
    frac = pos.astype(np.float64) @ np.linalg.inv(cell.astype(np.float64))
    fT_all = frac.T.astype(np.float32)                 # (3, N)

    # no separate self part: with invg_ii = 1/(2 sigma_i) the device's
    # eps-regularised (i==i, s=0) pairs give (erf(r invg) - erf(r c1))/r
    # -> (2/sqrt(pi))(1/(2 sigma_i) - 1/(sqrt2 eta))
    #  = 1/(sqrt(pi) sigma_i) - sqrt(2/pi)/eta, the analytic self term.

    in_maps = []
    for c in range(NCORES):
        lhsu = np.zeros((3, upc * P), np.float32)
        a2r = np.zeros((1, upc * P), np.float32)
        rbase = np.zeros((3, upc * P), np.float32)
        rshift = np.zeros((3, upc * SG), np.float32)
        s2l = np.ones((2, upc * P), np.float32)
        s2r = np.ones((2, upc * P), np.float32)
        q2c = np.zeros((P, upc), np.float32)
        qjr = np.zeros((upc, P), np.float32)
        for k in range(upc):
            unit = units[c * upc + k]
            if unit is None:
                continue   # zero-weight dummy; s2l/s2r stay 1 -> finite invg
            ti, tj, sg = unit
            wu = 1.0 if ti == tj else 2.0
            ai = a[ti * P:(ti + 1) * P]                # (128, 3)
            lhsu[:, k * P:(k + 1) * P] = ai.T
            a2r[0, k * P:(k + 1) * P] = a2[ti * P:(ti + 1) * P]
            aj = a[tj * P:(tj + 1) * P]                # (128, 3)
            # device builds -2b[s,j] = -2 a_j + -2 shift_s (s-major columns)
            rbase[:, k * P:(k + 1) * P] = -2.0 * aj.T
            rshift[:, k * SG:(k + 1) * SG] = \
                -2.0 * shifts[sg * SG:(sg + 1) * SG].T
            s2l[0, k * P:(k + 1) * P] = sig2[ti * P:(ti + 1) * P]
            s2r[1, k * P:(k + 1) * P] = sig2[tj * P:(tj + 1) * P]
            q2c[:, k] = q[ti * P:(ti + 1) * P] * np.float32(0.5 * COEF * wu)
            qjr[k] = q[tj * P:(tj + 1) * P]
        ksl = slice(c * KPC, (c + 1) * KPC)
        mTc = np.empty((3, 2 * KPC), np.float32)
        mTc[:, 0:KPC] = gk_pad[ksl].T.astype(np.float32)
        mTc[:, KPC:] = mTc[:, 0:KPC]
        w2c = np.broadcast_to(wk[ksl].astype(np.float32), (2, KPC)).copy()
        tens = {
            "lhsu": lhsu, "a2r": a2r, "rbase": rbase, "rshift": rshift,
            "s2l": s2l, "s2r": s2r,
            "q2c": q2c.astype(np.float16), "qjr": qjr,
            "mT": mTc, "fT": fT_all, "w2": w2c,
            "qmat16": q.reshape(NT, P).T.astype(np.float16),
        }
        pk32 = np.concatenate(
            [np.ascontiguousarray(tens[n]).ravel() for n, _, _ in _PACK32(upc)]
        )[None, :].astype(np.float32)
        pk16 = np.concatenate(
            [np.ascontiguousarray(tens[n]).ravel() for n, _, _ in _PACK16(upc)]
        )[None, :].astype(np.float16)
        in_maps.append({"pk32": pk32, "pk16": pk16})
    return in_maps, c1, upc


_runner_cache = {}


def _make_runner(nc, n_cores):
    """Build the jitted SPMD executable for ``nc`` ONCE.

    ``bass_utils.run_bass_kernel_spmd`` (axon path) creates a fresh closure
    and a fresh ``jax.jit`` on every call, so every invocation re-traces,
    re-lowers and re-runs ``compile_bir_kernel``/DVE-table generation
    (~200 ms of host work per call).  This mirrors its exact execution
    semantics (same ``_bass_exec_p`` bind, same shard_map layout, same
    donated zero-initialised outputs) but hoists all of that out of the
    per-call path: steady-state calls are just concat + dispatch + fetch.
    """
    import jax
    from concourse import bass2jax, mybir

    bass2jax.install_neuronx_cc_hook()
    if nc.dbg_addr is not None and nc.dbg_callbacks:
        raise RuntimeError("dbg callbacks unsupported in cached runner")
    partition_name = nc.partition_id_tensor.name if nc.partition_id_tensor else None
    dbg_name = nc.dbg_addr.name if nc.dbg_addr is not None else None

    in_names, in_specs_np, out_names, out_avals = [], [], [], []
    for alloc in nc.m.functions[0].allocations:
        if not isinstance(alloc, mybir.MemoryLocationSet):
            continue
        name = alloc.memorylocations[0].name
        if alloc.kind == "ExternalInput":
            if name != partition_name:
                in_names.append(name)
                in_specs_np.append((tuple(alloc.tensor_shape),
                                    mybir.dt.np(alloc.dtype)))
        elif alloc.kind == "ExternalOutput":
            out_names.append(name)
            out_avals.append(jax.core.ShapedArray(
                tuple(alloc.tensor_shape), mybir.dt.np(alloc.dtype)))
    n_params = len(in_names)
    n_outs = len(out_names)
    all_names = in_names + out_names + ([partition_name] if partition_name else [])
    donate = tuple(range(n_params, n_params + n_outs))

    def _body(*args):
        operands = list(args)
        if partition_name is not None:
            operands.append(bass2jax.partition_id_tensor())
        return tuple(bass2jax._bass_exec_p.bind(
            *operands, out_avals=tuple(out_avals), in_names=tuple(all_names),
            out_names=tuple(out_names), lowering_input_output_aliases=(),
            sim_require_finite=True, sim_require_nnan=True, nc=nc))

    devices = jax.devices()[:n_cores]
    assert len(devices) == n_cores
    mesh = bass2jax.Mesh(np.asarray(devices), ("core",))
    PS = bass2jax.PartitionSpec
    sharded = jax.jit(
        bass2jax.shard_map(_body, mesh=mesh,
                           in_specs=(PS("core"),) * (n_params + n_outs),
                           out_specs=(PS("core"),) * n_outs,
                           check_rep=False),
        donate_argnums=donate, keep_unused=True)
    # AOT-compile to skip the python pjit dispatch path (~3-8 ms/call)
    try:
        structs = [jax.ShapeDtypeStruct((n_cores * s[0], *s[1:]), dt)
                   for s, dt in in_specs_np]
        structs += [jax.ShapeDtypeStruct((n_cores * av.shape[0],
                                          *av.shape[1:]), av.dtype)
                    for av in out_avals]
        call = sharded.lower(*structs).compile()
    except Exception:
        call = sharded

    concat_bufs = None

    def run(in_maps):
        nonlocal concat_bufs
        if dbg_name is not None:
            in_maps = [{**m, dbg_name: np.zeros((1, 2), np.uint32)}
                       for m in in_maps]
        if concat_bufs is None:
            concat_bufs = [np.empty((n_cores * s[0], *s[1:]), dt)
                           for s, dt in in_specs_np]
        # reusing the host staging buffers is safe: the previous call's
        # output was fetched, so its input transfers have completed
        concat_in = []
        for buf, name in zip(concat_bufs, in_names):
            np.concatenate([np.asarray(m[name]) for m in in_maps],
                           axis=0, out=buf)
            concat_in.append(buf)
        concat_zeros = [
            np.zeros((n_cores * av.shape[0], *av.shape[1:]), av.dtype)
            for av in out_avals]
        out_arrs = call(*concat_in, *concat_zeros)
        outs_np = [np.asarray(o) for o in out_arrs]
        return [{name: outs_np[i].reshape(n_cores, *out_avals[i].shape)[c]
                 for i, name in enumerate(out_names)}
                for c in range(n_cores)]

    return run


def _run_spmd(nc, in_maps):
    """Run ``nc`` on 8 cores; cached-jit fast path with library fallback."""
    key = id(nc)
    run = _runner_cache.get(key)
    if run is None:
        try:
            run = _make_runner(nc, NCORES)
            _runner_cache[key] = run
        except Exception:
            run = None
    if run is not None:
        try:
            return run(in_maps)
        except Exception:
            _runner_cache.pop(key, None)
    from concourse import bass_utils
    res = bass_utils.run_bass_kernel_spmd(nc, in_maps,
                                          core_ids=list(range(NCORES)))
    return res.results


def kernel(pos, cell, charges, sigma_table, species_idx,
           nshift_real, nshift_recip):
    assert int(nshift_real) == NSHIFT_REAL and int(nshift_recip) == NSHIFT_RECIP, \
        "kernel compiled for nshift_real=1, nshift_recip=8"
    pos = np.asarray(pos)
    assert pos.shape == (N, 3)

    in_maps, c1, upc = prep_in_maps(pos, cell, charges, sigma_table,
                                    species_idx)
    nc = build_program(c1, upc)

    results = _run_spmd(nc, in_maps)
    e = np.float64(0.0)
    for i in range(NCORES):
        e += np.float64(results[i]["out"][0, 0])
    return np.array([[e]], dtype=np.float32)



# revision 53
# speedup vs baseline: 1.2040x; 1.0266x over previous
"""Trainium2 Bass kernel for the Ewald energy nn.Module.

Math restructuring (validated to ~2.6e-4 rel err against the jax reference
with the fp16 fast path):
  E = E_real + E_recip with the charge contraction folded in:
    E_real  = 0.5*COEF * sum_s sum_ij q_i q_j (erf(r/(sqrt2*gam_ij)) - erf(r/(sqrt2*eta)))/r
              over the symmetric half of the 6x6 grid of 128-atom tile pairs
              (off-diagonal pairs weighted 2x).  d^2 comes from Gram-matrix
              matmuls on the TensorEngine; atoms are pre-sorted along x so
              (tile-pair, x-shift-group) units whose x-gap exceeds the
              real-space cutoff are culled exactly (the reference masks
              r>cutoff).  With the true invgamma[i,i] = 1/(2 sigma_i), the
              eps-regularised (i==i, s=0) pairs reproduce the analytic
              self-energy term exactly, so no separate self part exists.
    E_recip = 0.5*COEF*(4pi/V) * sum_k w_k * (Sc_k^2 + Ss_k^2), structure
              factors via PE matmuls; only the ~1646 nonzero-weight k of the
              17^3 grid are computed.  theta is range-reduced via fractional
              coordinates and the (y + 1.5*2^23) - 1.5*2^23 round trick so
              Sin stays inside its [-pi, pi] table range.

Sharding: surviving real-space units are distributed round-robin over the 8
cores (UPC slots each, dummies zero-weighted); active k split 256 per core;
every core returns a scalar partial and the host sums the 8.

The run path is latency-dominated (axon tunnel RTT ~35 ms, measured
wall-clock), so the kernel minimises per-call host work and bytes shipped:
  * ~50 KB/core of inputs, shipped as just TWO flat buffers (one f32, one
    f16; per-array tunnel overhead and host allocations measurably widen
    the call).  Redundant tensors are rebuilt on device: the
    9-shift -2b rows from a per-unit base row + shift constants (broadcast
    add), |b|^2 = 0.25*colsum((-2b)^2) via a K=3 ones matmul, invgamma from
    sigma^2 vectors via a K=2 outer-sum matmul + Abs_reciprocal_sqrt, q_j
    broadcast over shifts in the final reduce, and the constant rows of the
    theta operands (memset row 0 - compute-engine partition ranges must
    start at 0/32/64).
  * d^2 accumulates K=3 (-2 a_i . b) + K=1 (+|a_i|^2) + K=1 (+|b|^2)
    matmuls in psum; the 1e-8 floor rides the activation bias so it is
    added AFTER the Gram cancellation (folding it into |a_i|^2 loses it to
    fp32 rounding and yields 1/sqrt(0) = inf on the diagonal).
  * _run_spmd caches an AOT-compiled jitted shard_map executable per
    program (the library helper re-traces, re-lowers and re-runs DVE-table
    generation on EVERY call, ~200 ms of host work).

ScalarEngine activations are phase-ordered (Abs_reciprocal_sqrt -> Erf ->
Sin) via emission order plus explicit scheduler edges through two tiny ACT
barrier copies, so each ACT table set loads exactly once.  The erf pipeline
runs in fp16 to unlock the DVE 2x perf mode.
"""
import math
import os
import sys
import numpy as np

_TRN_REPO = "/opt/trn_rl_repo"
if _TRN_REPO not in sys.path and os.path.isdir(_TRN_REPO):
    sys.path.insert(0, _TRN_REPO)

COEF = 14.399645478425668
N = 768
NT = 6            # 128-atom tiles
P = 128
NSHIFT_REAL = 1   # -> 27 shifts, 3 x-groups of 9
NSG = 3
SG = 9
UCOL = P * SG     # 1152 columns per unit
CH = 384          # matmul chunk
CHS = 512         # psum chunk stride (bank aligned)
NCH = 3
NCORES = 8
NSHIFT_RECIP = 8  # -> 17^3 = 4913 k-vectors; ~1646 carry weight
KPC = 256         # active k per core
_MAX_UNITS = 48   # provable upper bound on surviving units

_nc_cache = {}


def _PACK32(upc):
    """(name, rows, cols) layout of the flat f32 input buffer."""
    return [
        ("lhsu", 3, upc * P),          # a_i rows per unit
        ("a2r", 1, upc * P),           # |a_i|^2 per unit (single row)
        ("rbase", 3, upc * P),         # -2*a_j rows per unit
        ("rshift", 3, upc * SG),       # -2*shift rows per unit
        ("s2l", 2, upc * P),           # [sigma_i^2; 1] per unit
        ("s2r", 2, upc * P),           # [1; sigma_j^2] per unit
        ("qjr", upc, P),               # q_j per unit (broadcast over shifts)
        ("mT", 3, 2 * KPC),            # k rows; 24.0/24.25 row on device
        ("fT", 3, N),                  # frac rows; ones row on device
        ("w2", 2, KPC),
    ]


def _PACK16(upc):
    """(name, rows, cols) layout of the flat f16 input buffer."""
    return [
        ("q2c", P, upc),
        ("qmat16", P, NT),
    ]


def _tile_pairs():
    return [(ti, tj) for ti in range(NT) for tj in range(ti, NT)]


def _select_units(a, shifts, cutoff):
    """Cull (tilepair, shift-group) units whose x-gap exceeds the real-space
    cutoff.  Requires atoms sorted by x; sorted blocks guarantee at most 48
    survivors."""
    x = a[:, 0]
    lo = [x[t * P:(t + 1) * P].min() for t in range(NT)]
    hi = [x[t * P:(t + 1) * P].max() for t in range(NT)]
    units = []
    for (ti, tj) in _tile_pairs():
        for sg in range(NSG):
            keep = False
            for s in range(SG):
                sx = float(shifts[sg * SG + s, 0])
                d_lo = lo[tj] + sx - hi[ti]
                d_hi = hi[tj] + sx - lo[ti]
                if not (d_lo > cutoff or d_hi < -cutoff):
                    keep = True
                    break
            if keep:
                units.append((ti, tj, sg))
    assert len(units) <= _MAX_UNITS, f"{len(units)} units > {_MAX_UNITS}"
    return units


def build_program(c1, upc, dbg=False):
    """Build + compile the per-core Bass program (same on all cores).

    c1 = 1/(sqrt(2)*eta) is baked in as the erf scale constant; upc is the
    number of real-space unit slots per core.
    """
    key = ("nc", round(float(c1), 12), int(upc), bool(dbg))
    if key in _nc_cache:
        return _nc_cache[key]

    import concourse.bacc as bacc
    import concourse.tile as tile
    from concourse import mybir
    from concourse.tile import add_dep_helper

    AF = mybir.ActivationFunctionType
    OP = mybir.AluOpType
    f32 = mybir.dt.float32
    f16 = mybir.dt.float16

    nc = bacc.Bacc("TRN2", target_bir_lowering=False, debug=False)

    dt_in = {}

    def din(name, shape, dtype=f32):
        dt_in[name] = nc.dram_tensor(name, shape, dtype,
                                     kind="ExternalInput").ap()
        return dt_in[name]

    # all per-core tensors ride in two flat buffers (one per dtype) to cut
    # per-array tunnel overhead; device DMAs the slices into 2-D tiles
    specs32 = _PACK32(upc)
    specs16 = _PACK16(upc)
    n32 = sum(r * c for _, r, c in specs32)
    n16 = sum(r * c for _, r, c in specs16)
    din("pk32", [1, n32])
    din("pk16", [1, n16], f16)
    out_d = nc.dram_tensor("out", [1, 1], f32, kind="ExternalOutput").ap()
    if dbg:
        dbg_d = {
            "dbg_invg": nc.dram_tensor("dbg_invg", [P, upc * P], f16,
                                       kind="ExternalOutput").ap(),
            "dbg_rall": nc.dram_tensor("dbg_rall", [P, upc * UCOL], f16,
                                       kind="ExternalOutput").ap(),
            "dbg_arows": nc.dram_tensor("dbg_arows", [upc, UCOL], f32,
                                        kind="ExternalOutput").ap(),
            "dbg_rr": nc.dram_tensor("dbg_rr", [upc, 1], f32,
                                     kind="ExternalOutput").ap(),
            "dbg_rk": nc.dram_tensor("dbg_rk", [2, 1], f32,
                                     kind="ExternalOutput").ap(),
            "dbg_scs": nc.dram_tensor("dbg_scs", [2, KPC], f32,
                                      kind="ExternalOutput").ap(),
        }

    with tile.TileContext(nc) as tc:
        with tc.tile_pool(name="consts", bufs=1) as cp, \
             tc.tile_pool(name="rall", bufs=1) as rp, \
             tc.tile_pool(name="trig", bufs=6) as tp_, \
             tc.tile_pool(name="work", bufs=2) as wp, \
             tc.tile_pool(name="rbuild", bufs=2) as rbp, \
             tc.tile_pool(name="stage", bufs=2) as stp, \
             tc.tile_pool(name="scps", bufs=1, space="PSUM") as scp, \
             tc.tile_pool(name="bigps", bufs=2, space="PSUM") as bigp, \
             tc.tile_pool(name="thps", bufs=1, space="PSUM") as thp:

            # ---- unpack inputs to SBUF ----
            # fT/mT get a constant row 0 prepended on device (memset must
            # start at partition 0; DMA lands the data on partitions 1-3)
            sb = {}
            for specs, buf, dt in ((specs32, "pk32", f32),
                                   (specs16, "pk16", f16)):
                off = 0
                for name, r, c in specs:
                    if name in ("fT", "mT"):
                        t = cp.tile([4, c], dt, name=f"sb_{name}")
                        dst = t[1:4, :]
                    else:
                        t = cp.tile([r, c], dt, name=f"sb_{name}")
                        dst = t[:]
                    src = dt_in[buf][0:1, off:off + r * c] \
                        .rearrange("a (p f) -> (a p) f", f=c)
                    nc.sync.dma_start(dst, src)
                    sb[name] = t
                    off += r * c
            nc.vector.memset(sb["fT"][0:1, :], 1.0)
            nc.vector.memset(sb["mT"][0:1, 0:KPC], 24.0)
            nc.vector.memset(sb["mT"][0:1, KPC:], 24.25)

            beps = cp.tile([P, 1], f32)
            nc.vector.memset(beps[:], 1e-8)
            ones_t = cp.tile([P, 1], f32)
            nc.vector.memset(ones_t[:], 1.0)
            onesP = cp.tile([1, P], f32)           # K=1 lhs for the b^2 matmul
            nc.vector.memset(onesP[:], 1.0)
            onesC = cp.tile([1, CH], f32)          # K=1 rhs for the a^2 matmul
            nc.vector.memset(onesC[:], 1.0)
            ones3 = cp.tile([3, 1], f32)           # K=3 lhs for the b^2 colsum
            nc.vector.memset(ones3[:], 1.0)

            rall = rp.tile([P, upc * UCOL], f16)   # 1/r for all units
            invg_sb = cp.tile([P, upc * P], f16)   # 1/sqrt(2(si^2+sj^2))
            arows = cp.tile([upc, UCOL], f32)      # per-unit reduced rows
            scs = cp.tile([2, KPC], f32)           # [Sc; Ss] (squared later)
            scs_st = cp.tile([1, 2 * KPC], f32)    # partition-0 staging

            # ============ recip theta + range reduction (early) ============
            scargs = []
            RC = 12582912.0  # 1.5 * 2**23: (y + RC) - RC == round-to-nearest
            for t in range(NT):
                th = thp.tile([P, 2 * KPC], f32, tag="th")
                nc.tensor.matmul(th[:, 0:KPC], sb["fT"][:, t * P:(t + 1) * P],
                                 sb["mT"][:, 0:KPC], start=True, stop=True)
                nc.tensor.matmul(th[:, KPC:], sb["fT"][:, t * P:(t + 1) * P],
                                 sb["mT"][:, KPC:], start=True, stop=True)
                scarg = tp_.tile([P, 2 * KPC], f32, tag="scarg")
                rnd = stp.tile([P, 2 * KPC], f32, tag="rnd")
                nc.vector.tensor_scalar(rnd[:], th[:], RC, RC,
                                        OP.add, OP.subtract)
                nc.vector.tensor_tensor(scarg[:], th[:], rnd[:], OP.subtract)
                scargs.append(scarg)

            # ============ real part, phase 1: invg + d2 + 1/r ============
            # invg[i,j] = 1/sqrt(2(si^2+sj^2)) from a K=2 outer-sum matmul;
            # the true 1/(2*sigma_i) diagonal makes the eps-regularised
            # (i==i, s=0) pair reproduce the analytic self-energy term
            # exactly, so no separate self part is needed.
            sqrt_instrs = []
            for u in range(upc):
                ig = scp.tile([P, P], f32, tag="sc")
                nc.tensor.matmul(ig[:], sb["s2l"][:, u * P:(u + 1) * P],
                                 sb["s2r"][:, u * P:(u + 1) * P],
                                 start=True, stop=True)
                si = nc.scalar.activation(invg_sb[:, u * P:(u + 1) * P],
                                          ig[:], AF.Abs_reciprocal_sqrt,
                                          bias=0.0, scale=2.0)
                if sqrt_instrs:
                    add_dep_helper(si.ins, sqrt_instrs[-1].ins, sync=False,
                                   reason="sqrt unit order")
                sqrt_instrs.append(si)
            # -2b rows built on device from per-unit base + shift (broadcast
            # add), |b|^2 as 0.25 * colsum((-2b)^2) via a K=3 ones matmul;
            # then per 384-chunk: K=3 Gram (-2 a_i . b) + K=1 (+|a_i|^2)
            # + K=1 (+|b|^2) accumulated in psum, drained by one strided
            # Abs_reciprocal_sqrt with a pure-eps bias (the eps must be
            # added after the full cancellation or fp32 rounding eats it)
            for u in range(upc):
                rhs3 = rbp.tile([3, UCOL], f32, tag="rhs3")
                nc.vector.tensor_tensor(
                    rhs3[:].rearrange("p (s j) -> p s j", s=SG),
                    sb["rbase"][:, u * P:(u + 1) * P].unsqueeze(1)
                        .broadcast_to([3, SG, P]),
                    sb["rshift"][:, u * SG:(u + 1) * SG].unsqueeze(2)
                        .broadcast_to([3, SG, P]),
                    OP.add)
                sq3 = rbp.tile([3, UCOL], f32, tag="sq3")
                nc.vector.tensor_tensor(sq3[:], rhs3[:], rhs3[:], OP.mult)
                b2ps = bigp.tile([1, NCH * CHS], f32, tag="big")
                for ch in range(NCH):
                    nc.tensor.matmul(b2ps[0:1, ch * CHS:ch * CHS + CH],
                                     ones3[:], sq3[:, ch * CH:(ch + 1) * CH],
                                     start=True, stop=True)
                b2row = rbp.tile([1, UCOL], f32, tag="b2r")
                nc.vector.tensor_scalar_mul(
                    b2row[:].rearrange("p (c f) -> p c f", c=NCH),
                    b2ps[:].rearrange("p (c f) -> p c f", c=NCH)[:, :, 0:CH],
                    0.25)
                d2 = bigp.tile([P, NCH * CHS], f32, tag="big")
                for ch in range(NCH):
                    nc.tensor.matmul(d2[:, ch * CHS:ch * CHS + CH],
                                     sb["lhsu"][:, u * P:(u + 1) * P],
                                     rhs3[:, ch * CH:(ch + 1) * CH],
                                     start=True, stop=False)
                    nc.tensor.matmul(d2[:, ch * CHS:ch * CHS + CH],
                                     sb["a2r"][0:1, u * P:(u + 1) * P],
                                     onesC[:], start=False, stop=False)
                    nc.tensor.matmul(d2[:, ch * CHS:ch * CHS + CH],
                                     onesP[:],
                                     b2row[0:1, ch * CH:(ch + 1) * CH],
                                     start=False, stop=True)
                si = nc.scalar.activation(
                    rall[:, u * UCOL:(u + 1) * UCOL]
                        .rearrange("p (c f) -> p c f", c=NCH),
                    d2[:].rearrange("p (c f) -> p c f", c=NCH)[:, :, 0:CH],
                    AF.Abs_reciprocal_sqrt, bias=beps[:], scale=1.0)
                add_dep_helper(si.ins, sqrt_instrs[-1].ins, sync=False,
                               reason="sqrt unit order")
                sqrt_instrs.append(si)

            # ---- ACT barrier 1: abs_rsqrt -> erf ----
            bar1t = cp.tile([1, 1], f32)
            b1 = nc.scalar.copy(bar1t[:], ones_t[0:1, :])
            for s in sqrt_instrs:
                add_dep_helper(b1.ins, s.ins, sync=False, reason="act sqrt->erf")

            # ============ real part, phase 2: erf pipeline (fp16) ============
            erf_instrs = []
            for u in range(upc):
                rinv_u = rall[:, u * UCOL:(u + 1) * UCOL]
                H = UCOL // 2
                r_u = wp.tile([P, UCOL], f16, tag="r")
                with nc.allow_low_precision(reason="fp16 erf pipeline"):
                    nc.vector.reciprocal(r_u[:, 0:H], rinv_u[:, 0:H])
                    nc.vector.reciprocal(r_u[:, H:], rinv_u[:, H:])
                erf1 = wp.tile([P, UCOL], f16, tag="erf1")
                for hs in (slice(0, H), slice(H, UCOL)):
                    e1 = nc.scalar.activation(erf1[:, hs], r_u[:, hs], AF.Erf,
                                              bias=0.0, scale=float(c1))
                    add_dep_helper(e1.ins, b1.ins, sync=False,
                                   reason="act sqrt->erf")
                    erf_instrs.append(e1)
                arg2 = wp.tile([P, UCOL], f16, tag="arg2")
                # columns are s-major (col = s*128 + j): broadcast invg over s
                # with the unit-stride j innermost so DVE 2x mode applies
                invg_b = invg_sb[:, u * P:(u + 1) * P].unsqueeze(1) \
                    .broadcast_to([P, SG, P])
                nc.vector.tensor_tensor(
                    arg2[:].rearrange("p (s j) -> p s j", s=SG),
                    r_u[:].rearrange("p (s j) -> p s j", s=SG),
                    invg_b, OP.mult)
                erf2 = wp.tile([P, UCOL], f16, tag="erf2")
                e2 = nc.scalar.activation(erf2[:], arg2[:], AF.Erf,
                                          bias=0.0, scale=1.0)
                add_dep_helper(e2.ins, b1.ins, sync=False, reason="act sqrt->erf")
                erf_instrs.append(e2)
                # diff and valr in place (erf2 <- erf2-erf1 on GPSIMD,
                # erf1 <- diff*rinv on DVE)
                nc.gpsimd.tensor_tensor(erf2[:, 0:H], erf2[:, 0:H],
                                        erf1[:, 0:H], OP.subtract)
                nc.vector.tensor_tensor(erf2[:, H:], erf2[:, H:],
                                        erf1[:, H:], OP.subtract)
                nc.vector.tensor_tensor(erf1[:], erf2[:], rinv_u, OP.mult)
                ast = stp.tile([1, UCOL], f32, tag="ast")
                red = bigp.tile([1, NCH * CHS], f32, tag="big")
                for ch in range(NCH):
                    nc.tensor.matmul(red[0:1, ch * CHS:ch * CHS + CH],
                                     sb["q2c"][:, u:u + 1],
                                     erf1[:, ch * CH:(ch + 1) * CH],
                                     start=True, stop=True)
                nc.vector.tensor_copy(
                    ast[:].rearrange("p (c f) -> p c f", c=NCH),
                    red[:].rearrange("p (c f) -> p c f", c=NCH)[:, :, 0:CH])
                nc.sync.dma_start(arows[u:u + 1, :], ast[:])

            racc_r = cp.tile([upc, 1], f32)
            trash_r = cp.tile([upc, UCOL], f32)
            nc.vector.tensor_tensor(
                trash_r[:].rearrange("u (s j) -> u s j", s=SG),
                arows[:].rearrange("u (s j) -> u s j", s=SG),
                sb["qjr"][:].unsqueeze(1).broadcast_to([upc, SG, P]),
                OP.mult)
            nc.vector.tensor_reduce(racc_r[:], trash_r[:],
                                    axis=mybir.AxisListType.X, op=OP.add)

            # ---- ACT barrier 2: erf -> sin ----
            bar2t = cp.tile([1, 1], f32)
            b2 = nc.scalar.copy(bar2t[:], ones_t[0:1, :])
            for e in erf_instrs:
                add_dep_helper(b2.ins, e.ins, sync=False, reason="act erf->sin")

            # ============ reciprocal part ============
            # psum row: [Sc | Ss] accumulators in one bank
            scrow = scp.tile([1, 2 * KPC], f32, tag="sc")
            sncs = []
            for t in range(NT):
                snc = tp_.tile([P, 2 * KPC], f16, tag="snc")
                si = nc.scalar.activation(snc[:], scargs[t][:], AF.Sin,
                                          bias=0.0, scale=float(2 * math.pi))
                add_dep_helper(si.ins, b2.ins, sync=False, reason="act erf->sin")
                sncs.append(snc)
            for t in range(NT):
                nc.tensor.matmul(scrow[0:1, 0:KPC], sb["qmat16"][:, t:t + 1],
                                 sncs[t][:, KPC:],
                                 start=(t == 0), stop=(t == NT - 1))
            for t in range(NT):
                nc.tensor.matmul(scrow[0:1, KPC:], sb["qmat16"][:, t:t + 1],
                                 sncs[t][:, 0:KPC],
                                 start=(t == 0), stop=(t == NT - 1))
            nc.scalar.copy(scs_st[:], scrow[:])
            nc.sync.dma_start(scs[:], scs_st[:])

            sqk = cp.tile([2, KPC], f32)
            nc.vector.tensor_tensor(sqk[:], scs[:], scs[:], OP.mult)
            racc_k = cp.tile([2, 1], f32)
            trash_k = cp.tile([2, KPC], f32)
            nc.vector.tensor_tensor(trash_k[:], sqk[:], sb["w2"][:], OP.mult)
            nc.vector.tensor_reduce(racc_k[:], trash_k[:],
                                    axis=mybir.AxisListType.X, op=OP.add)

            # ============ combine ============
            # (no separate self part: the eps-regularised diagonal pairs of
            # the real part reproduce it exactly)
            sacc = cp.tile([P, 1], f32)
            nc.vector.memset(sacc[:], 0.0)
            nc.vector.tensor_tensor(sacc[0:upc, :], sacc[0:upc, :],
                                    racc_r[:], OP.add)
            nc.vector.tensor_tensor(sacc[0:2, :], sacc[0:2, :],
                                    racc_k[:], OP.add)
            fin = thp.tile([1, 1], f32, tag="th")
            nc.tensor.matmul(fin[:], sacc[:], ones_t[:], start=True, stop=True)
            outt = cp.tile([1, 1], f32)
            nc.vector.tensor_copy(outt[:], fin[:])
            nc.sync.dma_start(out_d[:], outt[:])
            if dbg:
                nc.sync.dma_start(dbg_d["dbg_invg"][:], invg_sb[:])
                nc.sync.dma_start(dbg_d["dbg_rall"][:], rall[:])
                nc.sync.dma_start(dbg_d["dbg_arows"][:], arows[:])
                nc.sync.dma_start(dbg_d["dbg_rr"][:], racc_r[:])
                nc.sync.dma_start(dbg_d["dbg_rk"][:], racc_k[:])
                nc.sync.dma_start(dbg_d["dbg_scs"][:], scs[:])

    nc.compile()
    _nc_cache[key] = nc
    return nc


def _shift_grid(n):
    r = np.arange(-n, n + 1, dtype=np.float64)
    g = np.stack(np.meshgrid(r, r, r, indexing="ij"), axis=-1)
    return g.reshape(-1, 3)


def prep_in_maps(pos, cell, charges, sigma_table, species_idx):
    """Host-side shard prep: returns (in_maps list of 8 dicts, c1, upc)."""
    pos = np.asarray(pos, np.float32)
    cell = np.asarray(cell, np.float32)
    if cell.ndim == 3:
        cell = cell[0]
    q = np.asarray(charges, np.float32).reshape(-1)
    sigma_table = np.asarray(sigma_table, np.float32)
    species_idx = np.asarray(species_idx).astype(np.int64)
    sigmas = sigma_table[species_idx]

    vol = abs(np.linalg.det(cell.astype(np.float64)))
    eta = (vol ** 2 / N) ** (1.0 / 6.0) / math.sqrt(2.0 * math.pi)
    cutoff_recip = math.sqrt(-2.0 * math.log(1e-8)) / eta
    cutoff_real = math.sqrt(-2.0 * math.log(1e-8)) * eta
    c1 = 1.0 / (math.sqrt(2.0) * eta)

    # sort atoms along x so the 128-atom tiles become x-slabs (enables exact
    # culling of far tile-pair/shift units)
    perm = np.argsort(pos[:, 0], kind="stable")
    pos = pos[perm]
    q = q[perm]
    sigmas = sigmas[perm]

    center = 0.5 * cell.astype(np.float64).sum(axis=0)
    a = (pos.astype(np.float64) - center).astype(np.float32)
    a2 = (a * a).sum(1).astype(np.float32)
    shifts = (_shift_grid(NSHIFT_REAL) @ cell.astype(np.float64)).astype(np.float32)

    sig2 = sigmas.astype(np.float32) ** 2

    units = _select_units(a, shifts, cutoff_real)
    upc = max(1, (len(units) + NCORES - 1) // NCORES)
    units = units + [None] * (NCORES * upc - len(units))

    # reciprocal k-grid: keep only k with nonzero weight (exact culling)
    gk = _shift_grid(NSHIFT_RECIP)                     # (4913, 3) float64
    recip = 2.0 * math.pi * np.linalg.inv(cell.astype(np.float64)).T
    ks_all = gk @ recip
    klen_all = np.linalg.norm(ks_all, axis=-1)
    kmask = (klen_all > 1e-8) & (klen_all < cutoff_recip)
    kidx = np.nonzero(kmask)[0]
    KTOT = NCORES * KPC
    assert len(kidx) <= KTOT, f"{len(kidx)} active k > {KTOT} slots"
    gk_pad = np.zeros((KTOT, 3), np.float64)
    gk_pad[: len(kidx)] = gk[kidx]
    wk = np.zeros(KTOT, np.float64)
    wk[: len(kidx)] = (np.exp(-0.5 * (eta * klen_all[kidx]) ** 2)
                       / klen_all[kidx] ** 2)
    wk = wk * (0.5 * COEF * 4.0 * math.pi / vol)
    frac = pos.astype(np.float64) @ np.linalg.inv(cell.astype(np.float64))
    fT_all = frac.T.astype(np.float32)                 # (3, N)

    # no separate self part: with invg_ii = 1/(2 sigma_i) the device's
    # eps-regularised (i==i, s=0) pairs give (erf(r invg) - erf(r c1))/r
    # -> (2/sqrt(pi))(1/(2 sigma_i) - 1/(sqrt2 eta))
    #  = 1/(sqrt(pi) sigma_i) - sqrt(2/pi)/eta, the analytic self term.

    in_maps = []
    for c in range(NCORES):
        lhsu = np.zeros((3, upc * P), np.float32)
        a2r = np.zeros((1, upc * P), np.float32)
        rbase = np.zeros((3, upc * P), np.float32)
        rshift = np.zeros((3, upc * SG), np.float32)
        s2l = np.ones((2, upc * P), np.float32)
        s2r = np.ones((2, upc * P), np.float32)
        q2c = np.zeros((P, upc), np.float32)
        qjr = np.zeros((upc, P), np.float32)
        for k in range(upc):
            unit = units[c * upc + k]
            if unit is None:
                continue   # zero-weight dummy; s2l/s2r stay 1 -> finite invg
            ti, tj, sg = unit
            wu = 1.0 if ti == tj else 2.0
            ai = a[ti * P:(ti + 1) * P]                # (128, 3)
            lhsu[:, k * P:(k + 1) * P] = ai.T
            a2r[0, k * P:(k + 1) * P] = a2[ti * P:(ti + 1) * P]
            aj = a[tj * P:(tj + 1) * P]                # (128, 3)
            # device builds -2b[s,j] = -2 a_j + -2 shift_s (s-major columns)
            rbase[:, k * P:(k + 1) * P] = -2.0 * aj.T
            rshift[:, k * SG:(k + 1) * SG] = \
                -2.0 * shifts[sg * SG:(sg + 1) * SG].T
            s2l[0, k * P:(k + 1) * P] = sig2[ti * P:(ti + 1) * P]
            s2r[1, k * P:(k + 1) * P] = sig2[tj * P:(tj + 1) * P]
            q2c[:, k] = q[ti * P:(ti + 1) * P] * np.float32(0.5 * COEF * wu)
            qjr[k] = q[tj * P:(tj + 1) * P]
        ksl = slice(c * KPC, (c + 1) * KPC)
        mTc = np.empty((3, 2 * KPC), np.float32)
        mTc[:, 0:KPC] = gk_pad[ksl].T.astype(np.float32)
        mTc[:, KPC:] = mTc[:, 0:KPC]
        w2c = np.broadcast_to(wk[ksl].astype(np.float32), (2, KPC)).copy()
        tens = {
            "lhsu": lhsu, "a2r": a2r, "rbase": rbase, "rshift": rshift,
            "s2l": s2l, "s2r": s2r,
            "q2c": q2c.astype(np.float16), "qjr": qjr,
            "mT": mTc, "fT": fT_all, "w2": w2c,
            "qmat16": q.reshape(NT, P).T.astype(np.float16),
        }
        pk32 = np.concatenate(
            [np.ascontiguousarray(tens[n]).ravel() for n, _, _ in _PACK32(upc)]
        )[None, :].astype(np.float32)
        pk16 = np.concatenate(
            [np.ascontiguousarray(tens[n]).ravel() for n, _, _ in _PACK16(upc)]
        )[None, :].astype(np.float16)
        in_maps.append({"pk32": pk32, "pk16": pk16})
    return in_maps, c1, upc


_runner_cache = {}


def _make_runner(nc, n_cores):
    """Build the jitted SPMD executable for ``nc`` ONCE.

    ``bass_utils.run_bass_kernel_spmd`` (axon path) creates a fresh closure
    and a fresh ``jax.jit`` on every call, so every invocation re-traces,
    re-lowers and re-runs ``compile_bir_kernel``/DVE-table generation
    (~200 ms of host work per call).  This mirrors its exact execution
    semantics (same ``_bass_exec_p`` bind, same shard_map layout, same
    donated zero-initialised outputs) but hoists all of that out of the
    per-call path: steady-state calls are just concat + dispatch + fetch.
    """
    import jax
    from concourse import bass2jax, mybir

    bass2jax.install_neuronx_cc_hook()
    if nc.dbg_addr is not None and nc.dbg_callbacks:
        raise RuntimeError("dbg callbacks unsupported in cached runner")
    partition_name = nc.partition_id_tensor.name if nc.partition_id_tensor else None
    dbg_name = nc.dbg_addr.name if nc.dbg_addr is not None else None

    in_names, in_specs_np, out_names, out_avals = [], [], [], []
    for alloc in nc.m.functions[0].allocations:
        if not isinstance(alloc, mybir.MemoryLocationSet):
            continue
        name = alloc.memorylocations[0].name
        if alloc.kind == "ExternalInput":
            if name != partition_name:
                in_names.append(name)
                in_specs_np.append((tuple(alloc.tensor_shape),
                                    mybir.dt.np(alloc.dtype)))
        elif alloc.kind == "ExternalOutput":
            out_names.append(name)
            out_avals.append(jax.core.ShapedArray(
                tuple(alloc.tensor_shape), mybir.dt.np(alloc.dtype)))
    n_params = len(in_names)
    n_outs = len(out_names)
    all_names = in_names + out_names + ([partition_name] if partition_name else [])
    donate = tuple(range(n_params, n_params + n_outs))

    def _body(*args):
        operands = list(args)
        if partition_name is not None:
            operands.append(bass2jax.partition_id_tensor())
        return tuple(bass2jax._bass_exec_p.bind(
            *operands, out_avals=tuple(out_avals), in_names=tuple(all_names),
            out_names=tuple(out_names), lowering_input_output_aliases=(),
            sim_require_finite=True, sim_require_nnan=True, nc=nc))

    devices = jax.devices()[:n_cores]
    assert len(devices) == n_cores
    mesh = bass2jax.Mesh(np.asarray(devices), ("core",))
    PS = bass2jax.PartitionSpec
    sharded = jax.jit(
        bass2jax.shard_map(_body, mesh=mesh,
                           in_specs=(PS("core"),) * (n_params + n_outs),
                           out_specs=(PS("core"),) * n_outs,
                           check_rep=False),
        donate_argnums=donate, keep_unused=True)
    # AOT-compile to skip the python pjit dispatch path (~3-8 ms/call)
    try:
        structs = [jax.ShapeDtypeStruct((n_cores * s[0], *s[1:]), dt)
                   for s, dt in in_specs_np]
        structs += [jax.ShapeDtypeStruct((n_cores * av.shape[0],
                                          *av.shape[1:]), av.dtype)
                    for av in out_avals]
        call = sharded.lower(*structs).compile()
    except Exception:
        call = sharded

    concat_bufs = None

    def run(in_maps):
        nonlocal concat_bufs
        if dbg_name is not None:
            in_maps = [{**m, dbg_name: np.zeros((1, 2), np.uint32)}
                       for m in in_maps]
        if concat_bufs is None:
            concat_bufs = [np.empty((n_cores * s[0], *s[1:]), dt)
                           for s, dt in in_specs_np]
        # reusing the host staging buffers is safe: the previous call's
        # output was fetched, so its input transfers have completed
        concat_in = []
        for buf, name in zip(concat_bufs, in_names):
            np.concatenate([np.asarray(m[name]) for m in in_maps],
                           axis=0, out=buf)
            concat_in.append(buf)
        concat_zeros = [
            np.zeros((n_cores * av.shape[0], *av.shape[1:]), av.dtype)
            for av in out_avals]
        out_arrs = call(*concat_in, *concat_zeros)
        outs_np = [np.asarray(o) for o in out_arrs]
        return [{name: outs_np[i].reshape(n_cores, *out_avals[i].shape)[c]
                 for i, name in enumerate(out_names)}
                for c in range(n_cores)]

    return run


def _run_spmd(nc, in_maps):
    """Run ``nc`` on 8 cores; cached-jit fast path with library fallback."""
    key = id(nc)
    run = _runner_cache.get(key)
    if run is None:
        try:
            run = _make_runner(nc, NCORES)
            _runner_cache[key] = run
        except Exception:
            run = None
    if run is not None:
        try:
            return run(in_maps)
        except Exception:
            _runner_cache.pop(key, None)
    from concourse import bass_utils
    res = bass_utils.run_bass_kernel_spmd(nc, in_maps,
                                          core_ids=list(range(NCORES)))
    return res.results


def kernel(pos, cell, charges, sigma_table, species_idx,
           nshift_real, nshift_recip):
    assert int(nshift_real) == NSHIFT_REAL and int(nshift_recip) == NSHIFT_RECIP, \
        "kernel compiled for nshift_real=1, nshift_recip=8"
    pos = np.asarray(pos)
    assert pos.shape == (N, 3)

    in_maps, c1, upc = prep_in_maps(pos, cell, charges, sigma_table,
                                    species_idx)
    nc = build_program(c1, upc)

    results = _run_spmd(nc, in_maps)
    e = np.float64(0.0)
    for i in range(NCORES):
        e += np.float64(results[i]["out"][0, 0])
    return np.array([[e]], dtype=np.float32)

